# revision 1
# baseline (speedup 1.0000x reference)
# Trainium2 Bass kernel for nn_LongformerSelfAttentionPegasus (B=2,S=4096,D=768,
# H=12,HD=64, window W=256 one-sided, G=128 global prefix tokens).
#
# Sharding (8 NeuronCores): sequence-parallel — core c handles batch c//4,
# query rows [1024*(c%4), 1024*(c%4+1)). Banded attention is fully local (the
# host ships a +/-W halo of the hidden states). The global-query attention
# (rows 0..G attend to all S tokens through the *_global projections) is
# token-parallel: each core computes exp-score partials (numerator+denominator
# via a ones-column on V) over its own 1024 tokens, and a [65*12, 128]
# AllReduce within each 4-core batch group completes the softmax. The final
# Dense + residual + LayerNorm are row-local, so no further communication.
#
# Perf structure (v1): band masking is a multiplicative {0,1} bf16 mask applied
# on VectorE after exp (no PE mask matmuls); the two heads sharing a partition
# tile are emitted as adjacent K=64 matmuls at base partitions 0/64 so they run
# concurrently in separate PE row groups (~1.9x); PV accumulates wide-N with a
# global-column start=True pass; emission is software-pipelined (projections
# and the global-attention section interleave with the first band pairs) so
# the PE never idles long enough to drop out of its warm clock state; ScalarE
# runs only Exp/Ln (bias adds, copies, LayerNorm stats and reciprocals live on
# VectorE/GpSimd) so the activation table never reloads.
import sys
import os as _os

for _p in ("/opt/trn_rl_repo",):
    if _p not in sys.path:
        sys.path.insert(0, _p)

import numpy as np
import ml_dtypes

import concourse.bass as bass
import concourse.bacc as bacc
import concourse.mybir as mybir
import concourse.tile as tile
from concourse import bass_utils

F32 = mybir.dt.float32
BF16 = mybir.dt.bfloat16
AF = mybir.ActivationFunctionType
ALU = mybir.AluOpType

B, S, D, H, HD = 2, 4096, 768, 12, 64
W, G = 256, 128
EPS = 1e-5
SCALE = 1.0 / np.sqrt(HD)

T = 1024                 # query rows per core
HALO = T + 2 * W         # 1536 banded kv rows per core
KT = G + HALO            # 1664 total kv rows (128 global + halo)
NBT = 12                 # band kcol tiles
NKP = KT // 128          # 13 v partition tiles

NQ = [128, 256, 384, 512, 640, 640, 640, 640, 512, 384, 256, 128]
MOFF = [0]
for _n in NQ:
    MOFF.append(MOFF[-1] + _n)
BAND_COLS = MOFF[-1]     # 5120
PTG0 = BAND_COLS         # ptg occupies cols [5120, 6144)
HCOLS = PTG0 + T         # 6144 score cols per head


def _lo(t):
    return max(0, t - 4)


# score segments: (t, gstart, width, qstart), split at the 512 psum-bank grid.
SEGS = []
for _t in range(13):
    if _t < 12:
        _g0, _nq, _l = MOFF[_t], NQ[_t], _lo(_t)
    else:
        _g0, _nq, _l = PTG0, T, 0
    _s = _g0
    while _s < _g0 + _nq:
        _e = min(_g0 + _nq, (_s // 512 + 1) * 512)
        SEGS.append((_t, _s, _e - _s, 128 * _l + (_s - _g0)))
        _s = _e
CHUNKS = [[sg for sg in SEGS if sg[1] // 1024 == c] for c in range(6)]

# PV out-column segments per band tile: (t, qc_start, width, rhs_off)
PVSEGS = []
for _t in range(12):
    _q0, _q1 = 128 * _lo(_t), 128 * _lo(_t) + NQ[_t]
    _s = _q0
    while _s < _q1:
        _e = min(_q1, (_s // 512 + 1) * 512)
        PVSEGS.append((_t, _s, _e - _s, MOFF[_t] + (_s - _q0)))
        _s = _e


def _emit(tc, dt):
    nc = tc.nc

    with (
        tc.tile_pool(name="const", bufs=1) as constp,
        tc.tile_pool(name="pers", bufs=1) as pers,
    ):
        # ---- ACT table warmup: Exp and Ln only, ever ----
        warm = constp.tile([1, 16], F32, tag="warm", name="warm")
        nc.vector.memset(warm[:], 1.0)
        nc.scalar.activation(warm[:], warm[:], AF.Exp)
        nc.scalar.activation(warm[:], warm[:], AF.Ln)

        # ---- constants ----
        maskm = constp.tile([128, BAND_COLS], BF16, tag="maskm", name="maskm")
        nc.sync.dma_start(maskm[:], dt["maskm"][:])
        msel = constp.tile([128, 2], F32, tag="msel", name="msel")
        nc.sync.dma_start(msel[:], dt["msel"][:])
        biasT = constp.tile([128, 24], F32, tag="biasT", name="biasT")
        nc.sync.dma_start(biasT[:], dt["biasT"][:])
        bias_t = {}
        for row, name in ((0, "bq"), (1, "bk"), (2, "bkg"), (3, "bqg")):
            bias_t[name] = [biasT[:, row * 6 + p:row * 6 + p + 1]
                            for p in range(6)]
        vrow = {}
        for row, name in ((0, "bv"), (1, "bvg"), (2, "gam"), (3, "bet")):
            t_ = constp.tile([128, D], BF16, tag=name, name=name)
            nc.sync.dma_start(t_[:], dt["vrep"][row])
            vrow[name] = t_

        # ---- persistent activation storage (bf16) ----
        kT = [pers.tile([128, KT], BF16, tag=f"kT{p}", name=f"kT{p}") for p in range(6)]
        qT = [pers.tile([128, T], BF16, tag=f"qT{p}", name=f"qT{p}") for p in range(6)]
        kgfT = [pers.tile([128, T], BF16, tag=f"kgfT{p}", name=f"kgfT{p}") for p in range(6)]
        qgT = [pers.tile([128, G], BF16, tag=f"qgT{p}", name=f"qgT{p}") for p in range(6)]
        vsb = [pers.tile([128, H * (HD + 1)], BF16, tag=f"v{p}", name=f"v{p}") for p in range(NKP)]
        vgf = [pers.tile([128, H * (HD + 1)], BF16, tag=f"vg{p}", name=f"vg{p}") for p in range(8)]
        ctxT = [pers.tile([128, T], BF16, tag=f"ctxT{p}", name=f"ctxT{p}") for p in range(6)]
        ogsb = pers.tile([128, H * G], BF16, tag="ogsb", name="ogsb")
        ogred = pers.tile([128, H * G], BF16, tag="ogred", name="ogred")
        ogd_cm = tc.tile_pool(name="ogdram", bufs=1, space="DRAM")
        ogd = ogd_cm.__enter__()
        og_in = ogd.tile([128, H * G], BF16, tag="og_in", name="og_in")
        og_out = ogd.tile([128, H * G], BF16, tag="og_out", name="og_out")

        nc.gpsimd.memset(ogsb[:], 0.0)
        for p in range(NKP):
            nc.gpsimd.memset(vsb[p][:], 1.0)
        for p in range(8):
            nc.gpsimd.memset(vgf[p][:], 1.0)

        with tc.tile_pool(name="ptp", bufs=2) as ptp:
            pts = {}

            def pt_tile():
                return ptp.tile([128, 2 * HCOLS], BF16, tag="pt", name="pt")

            with (
                tc.tile_pool(name="xw", bufs=2) as xw,
                tc.tile_pool(name="psA", bufs=2, space="PSUM") as psA,
                tc.tile_pool(name="ptgp", bufs=2) as ptgp,
                tc.tile_pool(name="xtp", bufs=1) as xtp,
            ):
                xT = [xtp.tile([128, KT], BF16, tag=f"xT{p}", name=f"xT{p}")
                      for p in range(6)]
                for p in range(6):
                    nc.sync.dma_start(xT[p][:], dt["xT"][128 * p:128 * p + 128, :])

                def load_w(widx):
                    tiles = []
                    for k in range(6):
                        t_ = xw.tile([128, D], BF16, tag=f"w{k}", name=f"w{k}")
                        nc.sync.dma_start(t_[:], dt["w"][widx, 128 * k:128 * k + 128, :])
                        tiles.append(t_)
                    return tiles

                def projT_group(wt, out_tiles, bias, xcol0, m, nn, gw):
                    ps = psA.tile([128, 1024], F32, tag="pa", name="pa")
                    for k in range(6):
                        p0 = 0
                        while p0 < gw:
                            pw = min(512, gw - p0)
                            nc.tensor.matmul(
                                ps[:, p0:p0 + pw],
                                wt[k][:, 128 * m:128 * m + 128],
                                xT[k][:, xcol0 + nn + p0:xcol0 + nn + p0 + pw],
                                start=(k == 0), stop=(k == 5))
                            p0 += pw
                    nc.vector.tensor_scalar(
                        out_tiles[m][:, nn:nn + gw], ps[:, :gw],
                        bias[m][:], None, ALU.add)

                def proj_T(wt, out_tiles, bias, xcol0, ncols):
                    for m in range(6):
                        nn = 0
                        while nn < ncols:
                            gw = min(1024, ncols - nn)
                            projT_group(wt, out_tiles, bias, xcol0, m, nn, gw)
                            nn += gw

                def projN_group(wt, out_tiles, brow, xcol0, m):
                    ps = psA.tile([128, 1024], F32, tag="pa", name="pa")
                    for k in range(6):
                        for p0, pw in ((0, 512), (512, 256)):
                            nc.tensor.matmul(
                                ps[:, p0:p0 + pw],
                                xT[k][:, xcol0 + 128 * m:xcol0 + 128 * m + 128],
                                wt[k][:, p0:p0 + pw],
                                start=(k == 0), stop=(k == 5))
                    ov = out_tiles[m][:].rearrange("p (h e) -> p h e", e=HD + 1)
                    for n0, nw in ((0, 512), (512, 256)):
                        h0 = n0 // HD
                        nh = nw // HD
                        nc.vector.tensor_tensor(
                            ov[:, h0:h0 + nh, :HD],
                            ps[:, n0:n0 + nw].rearrange("p (h e) -> p h e", e=HD),
                            vrow[brow][:, n0:n0 + nw]
                            .rearrange("p (h e) -> p h e", e=HD),
                            ALU.add)

                def qk_chunk(pr, pt, c, pool):
                    pss = [pool.tile([128, 1024], F32, tag="pa", name="pa")
                           for _ in (0, 1)]
                    for (t, gs, w_, qs) in CHUNKS[c]:
                        for half in (0, 1):
                            r0, r1 = 64 * half, 64 * half + 64
                            lt = (kT[pr][r0:r1, 0:G] if t == 12 else
                                  kT[pr][r0:r1, G + 128 * t:G + 128 * t + 128])
                            nc.tensor.matmul(
                                pss[half][:, gs - 1024 * c:gs - 1024 * c + w_],
                                lt, qT[pr][r0:r1, qs:qs + w_],
                                start=True, stop=True, skip_group_check=True)
                    mw = min(BAND_COLS - 1024 * c, 1024)  # mask cols in chunk
                    for half in (0, 1):
                        b = HCOLS * half + 1024 * c
                        nc.scalar.activation(
                            pt[:, b:b + 1024], pss[half][:], AF.Exp)
                        if mw > 0:
                            nc.vector.tensor_tensor(
                                pt[:, b:b + mw], pt[:, b:b + mw],
                                maskm[:, 1024 * c:1024 * c + mw], ALU.mult)

                def ptog_pr(pr, ptgsb):
                    for grp in (0, 1):
                        ps = psA.tile([128, 1024], F32, tag="pa", name="pa")
                        for tt in range(4 * grp, 4 * grp + 4):
                            for half in (0, 1):
                                r0, r1 = 64 * half, 64 * half + 64
                                nc.tensor.matmul(
                                    ps[:, 512 * half + 128 * (tt - 4 * grp):
                                       512 * half + 128 * (tt - 4 * grp) + 128],
                                    kgfT[pr][r0:r1, 128 * tt:128 * tt + 128],
                                    qgT[pr][r0:r1, :],
                                    start=True, stop=True,
                                    skip_group_check=True)
                        nc.scalar.activation(
                            ptgsb[:, 1024 * grp:1024 * grp + 1024],
                            ps[:], AF.Exp)

                def ognum_head(pr, half, ptgsb):
                    h = 2 * pr + half
                    ps = psA.tile([128, 1024], F32, tag="pa", name="pa")
                    for tt in range(8):
                        nc.tensor.matmul(
                            ps[:65, :G],
                            vgf[tt][:, (HD + 1) * h:(HD + 1) * h + HD + 1],
                            ptgsb[:, 1024 * (tt // 4) + 512 * half
                                  + 128 * (tt % 4):
                                  1024 * (tt // 4) + 512 * half
                                  + 128 * (tt % 4) + 128],
                            start=(tt == 0), stop=(tt == 7))
                    nc.vector.tensor_copy(ogsb[:65, G * h:G * h + G],
                                          ps[:65, :G])

                # ---- emission: projections pipelined with pairs 0/1 ----
                wk = load_w(1)
                proj_T(wk, kT, bias_t["bk"], 0, KT)
                wq = load_w(0)
                proj_T(wq, qT, bias_t["bq"], G + W, T)
                wv = load_w(2)
                for m in range(NKP):
                    projN_group(wv, vsb, "bv", 0, m)

                # pair0 QK chunks interleaved with kgf/qg/vgf projections
                pts[0] = pt_tile()
                wkg = load_w(3)
                wqg = load_w(5)
                wvg = load_w(4)
                projq = []
                for m in range(6):
                    projq.append(("T", wkg, kgfT, bias_t["bkg"], G + W, m, 0, 1024))
                projq.append(("T", wqg, qgT, bias_t["bqg"], 0, 0, 0, G))
                for m in range(1, 6):
                    projq.append(("T", wqg, qgT, bias_t["bqg"], 0, m, 0, G))
                for m in range(8):
                    projq.append(("N", wvg, vgf, "bvg", G + W, m))
                ci = 0
                for i, job in enumerate(projq):
                    if job[0] == "T":
                        _, wt, ot, bi, xc, m, nn, gw = job
                        projT_group(wt, ot, bi, xc, m, nn, gw)
                    else:
                        _, wt, ot, br, xc, m = job
                        projN_group(wt, ot, br, xc, m)
                    if i % 2 == 1 and ci < 6:
                        qk_chunk(0, pts[0], ci, psA)
                        ci += 1
                while ci < 6:
                    qk_chunk(0, pts[0], ci, psA)
                    ci += 1

                # ptog + og-num per pr, interleaved with pair1 QK chunks
                pts[1] = pt_tile()
                ci = 0
                for pr in range(6):
                    ptgsb = ptgp.tile([128, 2048], BF16, tag="ptgsb",
                                      name="ptgsb")
                    ptog_pr(pr, ptgsb)
                    ognum_head(pr, 0, ptgsb)
                    ognum_head(pr, 1, ptgsb)
                    if ci < 6:
                        qk_chunk(1, pts[1], ci, psA)
                        ci += 1
                while ci < 6:
                    qk_chunk(1, pts[1], ci, psA)
                    ci += 1

                nc.sync.dma_start(og_in[:], ogsb[:])

            # ---- band: PV(pr) interleaved with QK(pr+2) ----
            with (
                tc.tile_pool(name="psB", bufs=2, space="PSUM") as psB,
                tc.tile_pool(name="psC", bufs=2, space="PSUM") as psC,
                tc.tile_pool(name="nrm", bufs=2) as nrm,
            ):
                def pv_batch(pr, half, pt, ps, seglist):
                    b = HCOLS * half
                    h = 2 * pr + half
                    vcol = slice((HD + 1) * h, (HD + 1) * h + HD + 1)
                    for kind, qs, w_, ro, t in seglist:
                        if kind == "g":
                            nc.tensor.matmul(
                                ps[:, qs:qs + w_], vsb[0][:, vcol],
                                pt[:, b + PTG0 + ro:b + PTG0 + ro + w_],
                                start=True, stop=False)
                        else:
                            isstop = (t == 7 and qs == 384) or t == 11
                            nc.tensor.matmul(
                                ps[:, qs:qs + w_], vsb[1 + t][:, vcol],
                                pt[:, b + ro:b + ro + w_],
                                start=False, stop=isstop,
                                skip_group_check=not isstop)

                # PV work for one head: global cols first (start), then bands
                PVLIST = ([("g", 0, 512, 0, -1), ("g", 512, 512, 512, -1)]
                          + [("b", qs, w_, ro, t)
                             for (t, qs, w_, ro) in PVSEGS])

                def norm_head(pr, half, ps, dstage):
                    dinv = nrm.tile([1, T], F32, tag=f"dinv{half}",
                                    name=f"dinv{half}", bufs=1)
                    nc.vector.reciprocal_approx_fast(dinv[:], dstage[:])
                    invb = nrm.tile([HD, T], F32, tag=f"invb{half}",
                                    name=f"invb{half}", bufs=1)
                    nc.gpsimd.partition_broadcast(invb[:], dinv[0:1, :])
                    nc.vector.tensor_tensor(
                        ctxT[pr][64 * half:64 * half + HD, :],
                        ps[:HD, :], invb[:], ALU.mult)

                for pr in range(6):
                    # QK of pair pr+2 chunks interleave with PV of pair pr
                    qkc = list(range(6)) if pr + 2 <= 5 else []
                    if qkc:
                        pts[pr + 2] = pt_tile()
                    psh = []
                    dsth = []
                    for half in (0, 1):
                        ps = psB.tile([65, 1024], F32, tag="pv", name="pv")
                        psh.append(ps)
                        dsth.append(nrm.tile([1, T], F32, tag=f"dstage{half}",
                                             name=f"dstage{half}"))
                    # split PVLIST into 3 batches; alternate h0/h1 per seg
                    nb = 3
                    bsz = (len(PVLIST) + nb - 1) // nb
                    for j in range(nb):
                        if qkc and j < len(qkc):
                            qk_chunk(pr + 2, pts[pr + 2], qkc[j], psC)
                        for seg in PVLIST[j * bsz:(j + 1) * bsz]:
                            for half in (0, 1):
                                pv_batch(pr, half, pts[pr], psh[half], [seg])
                    for j in range(nb, 6):
                        if qkc and j < len(qkc):
                            qk_chunk(pr + 2, pts[pr + 2], qkc[j], psC)
                    for half in (0, 1):
                        nc.vector.tensor_copy(dsth[half][:],
                                              psh[half][64:65, :])
                        norm_head(pr, half, psh[half], dsth[half])

                # AllReduce now: all norm broadcasts are already queued on
                # GpSimd, so the collective no longer blocks PV rotation.
                if _os.environ.get("NO_CC") == "1":
                    nc.sync.dma_start(og_out[:], og_in[:])
                else:
                    nc.gpsimd.collective_compute(
                        "AllReduce", ALU.add,
                        replica_groups=[[0, 1, 2, 3], [4, 5, 6, 7]],
                        ins=[og_in.opt()], outs=[og_out.opt()])
                nc.sync.dma_start(ogred[:], og_out[:])

                # ---- fold the AllReduced global-attention output ----
                FHALF = 6 * G
                for fh in (0, 1):
                    fstage = nrm.tile([1, FHALF], F32, tag="fstage",
                                      name="fstage", bufs=1)
                    nc.vector.tensor_copy(
                        fstage[:], ogred[64:65, FHALF * fh:FHALF * fh + FHALF])
                    finv = nrm.tile([1, FHALF], F32, tag="finv", name="finv",
                                    bufs=1)
                    nc.vector.reciprocal_approx_fast(finv[:], fstage[:])
                    finvb = nrm.tile([HD, FHALF], F32, tag="finvb",
                                     name="finvb", bufs=1)
                    nc.gpsimd.partition_broadcast(finvb[:], finv[:])
                    for h in range(6 * fh, 6 * fh + 6):
                        pr, half = h // 2, h % 2
                        r0, r1 = 64 * half, 64 * half + 64
                        ogt = nrm.tile([128, G], BF16, tag="ogt", name="ogt")
                        nc.gpsimd.tensor_tensor(
                            ogt[r0:r1, :], ogred[:HD, G * h:G * h + G],
                            finvb[:, G * h - FHALF * fh:
                                  G * h - FHALF * fh + G],
                            ALU.mult)
                        nc.vector.tensor_scalar_mul(
                            ctxT[pr][r0:r1, :G], ctxT[pr][r0:r1, :G],
                            msel[r0:r1, 1:2])
                        nc.vector.scalar_tensor_tensor(
                            ctxT[pr][r0:r1, :G], ogt[r0:r1, :],
                            msel[r0:r1, 0:1],
                            ctxT[pr][r0:r1, :G], ALU.mult, ALU.add)

        # ---- output Dense + residual + LayerNorm (two-pass, batched act) ----
        with (
            tc.tile_pool(name="wo", bufs=1) as wop,
            tc.tile_pool(name="ln", bufs=2) as lnp,
            tc.tile_pool(name="psD", bufs=2, space="PSUM") as psD,
        ):
            wo = []
            for k in range(6):
                t_ = wop.tile([128, D], BF16, tag=f"wo{k}", name=f"wo{k}")
                nc.sync.dma_start(t_[:], dt["w"][6, 128 * k:128 * k + 128, :])
                wo.append(t_)
            epst = wop.tile([128, 1], F32, tag="epst", name="epst")
            nc.gpsimd.memset(epst[:], EPS)
            sumsq = wop.tile([128, 8], F32, tag="sumsq", name="sumsq")
            istd = wop.tile([128, 8], F32, tag="istd", name="istd")
            ycs = [wop.tile([128, D], F32, tag=f"yc{m}", name=f"yc{m}")
                   for m in range(8)]
            lnv = wop.tile([128, 8], F32, tag="lnv", name="lnv")

            def ln_pass1(m):
                ys = lnp.tile([128, D], F32, tag="ys", name="ys")
                rs = lnp.tile([128, D], F32, tag="rs", name="rs")
                nc.sync.dma_start(rs[:], dt["res"][128 * m:128 * m + 128, :])
                sums = lnp.tile([128, 2], F32, tag="sums", name="sums")
                ps = psD.tile([128, 1024], F32, tag="pd", name="pd")
                for k in range(6):
                    for n0, nw in ((0, 512), (512, 256)):
                        nc.tensor.matmul(
                            ps[:, n0:n0 + nw],
                            ctxT[k][:, 128 * m:128 * m + 128],
                            wo[k][:, n0:n0 + nw],
                            start=(k == 0), stop=(k == 5))
                nc.vector.scalar_tensor_tensor(
                    ys[:], ps[:, :D], 1.0, rs[:], ALU.mult, ALU.add,
                    accum_out=sums[:, 0:1])
                negmean = lnp.tile([128, 1], F32, tag="negmean", name="negmean")
                nc.vector.tensor_scalar_mul(negmean[:], sums[:, 0:1], -1.0 / D)
                nc.vector.tensor_scalar(ycs[m][:], ys[:], negmean[:], None,
                                        ALU.add)
                nc.vector.scalar_tensor_tensor(ys[:], ycs[m][:], 1.0, ycs[m][:],
                                               ALU.mult, ALU.mult,
                                               accum_out=sumsq[:, m:m + 1])

            def ln_pass2(m):
                yo = lnp.tile([128, D], F32, tag="yo", name="yo")
                nc.vector.scalar_tensor_tensor(
                    yo[:], ycs[m][:], istd[:, m:m + 1], vrow["gam"][:],
                    ALU.mult, ALU.mult)
                nc.vector.tensor_tensor(
                    yo[:], yo[:], vrow["bet"][:], ALU.add)
                nc.sync.dma_start(dt["y"][128 * m:128 * m + 128, :], yo[:])

            batches = [[1, 2, 3, 4], [5, 6, 7, 0]]
            for m in batches[0]:
                ln_pass1(m)
            nc.scalar.activation(lnv[:, 1:5], sumsq[:, 1:5], AF.Ln,
                                 bias=epst[:], scale=1.0 / D)
            nc.scalar.activation(istd[:, 1:5], lnv[:, 1:5], AF.Exp, scale=-0.5)
            for m in batches[1]:
                ln_pass1(m)
            for m in batches[0]:
                ln_pass2(m)
            nc.scalar.activation(lnv[:, 5:8], sumsq[:, 5:8], AF.Ln,
                                 bias=epst[:], scale=1.0 / D)
            nc.scalar.activation(istd[:, 5:8], lnv[:, 5:8], AF.Exp, scale=-0.5)
            nc.scalar.activation(lnv[:, 0:1], sumsq[:, 0:1], AF.Ln,
                                 bias=epst[:], scale=1.0 / D)
            nc.scalar.activation(istd[:, 0:1], lnv[:, 0:1], AF.Exp, scale=-0.5)
            for m in batches[1]:
                ln_pass2(m)
        ogd_cm.__exit__(None, None, None)


def build_nc():
    nc = bacc.Bacc(trn_type="TRN2", num_devices=8)
    dt = {
        "xT": nc.dram_tensor("xT", [D, KT], BF16, kind="ExternalInput"),
        "w": nc.dram_tensor("w", [7, D, D], BF16, kind="ExternalInput"),
        "res": nc.dram_tensor("res", [T, D], F32, kind="ExternalInput"),
        "maskm": nc.dram_tensor("maskm", [128, BAND_COLS], BF16,
                                kind="ExternalInput"),
        "msel": nc.dram_tensor("msel", [128, 2], F32, kind="ExternalInput"),
        "vrep": nc.dram_tensor("vrep", [4, 128, D], BF16, kind="ExternalInput"),
        "biasT": nc.dram_tensor("biasT", [128, 24], F32, kind="ExternalInput"),
        "y": nc.dram_tensor("y", [T, D], F32, kind="ExternalOutput"),
    }
    with tile.TileContext(nc) as tc:
        _emit(tc, dt)
    nc.compile()
    return nc


def host_inputs(inputs):
    """Build the 8 per-core input maps from the full problem inputs."""
    hs = np.asarray(inputs["hidden_states"], np.float32)
    assert hs.shape == (B, S, D)
    bf = lambda a: np.ascontiguousarray(np.asarray(a, np.float32)).astype(
        ml_dtypes.bfloat16)
    f32 = lambda a: np.ascontiguousarray(np.asarray(a, np.float32))

    wstack = np.stack([
        np.asarray(inputs["Wq"], np.float32) * SCALE,
        np.asarray(inputs["Wk"], np.float32),
        np.asarray(inputs["Wv"], np.float32),
        np.asarray(inputs["Wkg"], np.float32),
        np.asarray(inputs["Wvg"], np.float32),
        np.asarray(inputs["Wqg"], np.float32) * SCALE,
        np.asarray(inputs["Wo"], np.float32),
    ])
    vecs = np.stack([
        np.asarray(inputs["bq"], np.float32) * SCALE,
        np.asarray(inputs["bk"], np.float32),
        np.asarray(inputs["bkg"], np.float32),
        np.asarray(inputs["bqg"], np.float32) * SCALE,
    ])
    bo = np.asarray(inputs["bo"], np.float32)
    biasT = np.zeros((128, 24), np.float32)
    for row in range(4):
        for p in range(6):
            biasT[:, row * 6 + p] = vecs[row, 128 * p:128 * p + 128]
    vrep = bf(np.broadcast_to(
        np.stack([
            np.asarray(inputs["bv"], np.float32),
            np.asarray(inputs["bvg"], np.float32),
            np.asarray(inputs["ln_gamma"], np.float32),
            np.asarray(inputs["ln_beta"], np.float32),
        ])[:, None, :], (4, 128, D)))

    w_bf = bf(wstack)

    in_maps = []
    for c in range(8):
        b, j = c // 4, c % 4
        r0 = j * T
        x = hs[b]
        xp = np.zeros((S + 2 * W, D), np.float32)
        xp[W:W + S] = x
        x_kv = np.concatenate([x[:G], xp[r0:r0 + HALO]], axis=0)  # [1664, D]
        xT = bf(x_kv.T)
        res = f32(x[r0:r0 + T] + bo)

        # multiplicative band mask, {0,1} bf16, MOFF layout
        mask = np.zeros((128, BAND_COLS), np.float32)
        for t in range(NBT):
            lo = _lo(t)
            nq = NQ[t]
            jj = np.arange(128 * t, 128 * t + 128)[:, None]
            ii = np.arange(lo * 128, lo * 128 + nq)[None, :]
            kpos = r0 - W + jj
            valid = ((jj - ii >= 0) & (jj - ii <= 2 * W)
                     & (kpos >= G) & (kpos < S))
            mask[:, MOFF[t]:MOFF[t] + nq] = valid.astype(np.float32)

        m = 1.0 if j == 0 else 0.0
        msel = np.zeros((128, 2), np.float32)
        msel[:, 0] = m
        msel[:, 1] = 1.0 - m

        in_maps.append({
            "xT": xT, "w": w_bf, "res": res,
            "maskm": bf(mask), "msel": f32(msel),
            "vrep": vrep, "biasT": biasT,
        })
    return in_maps


_NC_CACHE = {}


def _get_nc():
    if "nc" not in _NC_CACHE:
        _NC_CACHE["nc"] = build_nc()
    return _NC_CACHE["nc"]


def kernel(**inputs) -> np.ndarray:
    # sanity-check the fixed global-attention pattern this kernel hardcodes
    iga = np.asarray(inputs["is_index_global_attn"])
    assert iga.shape == (B, S)
    expect = np.broadcast_to(np.arange(S) < G, (B, S))
    assert np.array_equal(iga, expect), "kernel hardcodes a G=128 prefix"
    am = np.asarray(inputs["attention_mask"], np.float32)
    assert np.all(am == 0.0), "kernel assumes no key-padding mask"

    nc = _get_nc()
    in_maps = host_inputs(inputs)
    res = bass_utils.run_bass_kernel_spmd(nc, in_maps, core_ids=list(range(8)))
    outs = res.results if hasattr(res, "results") else res
    y = np.zeros((B, S, D), np.float32)
    for c in range(8):
        b, j = c // 4, c % 4
        y[b, j * T:(j + 1) * T] = outs[c]["y"]
    return y


if __name__ == "__main__":
    nc = build_nc()
    print("build ok; instructions:",
          sum(len(bb.instructions) for bb in nc.main_func.blocks))



# revision 11
# speedup vs baseline: 1.1495x; 1.1495x over previous
# Trainium2 Bass kernel for nn_LongformerSelfAttentionPegasus (B=2,S=4096,D=768,
# H=12,HD=64, window W=256 one-sided, G=128 global prefix tokens).
#
# Sharding (8 NeuronCores): sequence-parallel — core c handles batch c//4,
# query rows [1024*(c%4), 1024*(c%4+1)). Banded attention is fully local (the
# host ships a +/-W halo of the hidden states). The global-query attention
# (rows 0..G attend to all S tokens through the *_global projections) is
# token-parallel: each core computes exp-score partials (numerator+denominator
# via a 1/64-column on V) over its own 1024 tokens, and a [65, H*G] bf16
# AllReduce within each 4-core batch group completes the softmax. The final
# Dense + residual + LayerNorm are row-local, so no further communication.
#
# Perf structure (v2):
#  - All seven projections run in fp8e4m3 with the DoubleRow perf mode
#    (K=256 per matmul at 0.5 cycles/col). Weights are quantized per-tensor
#    (x64, x512 for the pre-scaled q/qg) which costs <1e-3 rel err; the
#    activations are double-pumped (x = xa + xb residual split) on the
#    q/k/kg/qg paths where score error is exp-amplified, single-pumped on
#    v/vg. The context is written as fp8 (x64) so the output Dense also runs
#    DoubleRow.
#  - Band masking is a multiplicative {0,1} bf16 mask applied on VectorE
#    after exp; two heads share a partition tile and are emitted as adjacent
#    K=64 matmuls at base partitions 0/64 so they run concurrently in
#    separate PE row groups; PV accumulates wide-N with a global-column
#    start=True pass; softmax denominators ride a 1/64 ones-column so the
#    reciprocal (read straight out of PSUM) is already the x64 fp8 scale.
#  - The AllReduce is emitted before the band loop (trigger on GpSimd as
#    soon as the numerators' DMA lands) and its consumers are isolated: the
#    folded global rows live in separate ctxG tiles feeding only the m=0
#    output-Dense tile, which is emitted dead last together with the fold,
#    so a slow collective can never stall the band pipeline or the other
#    7/8 of the Dense+LayerNorm.
#  - LayerNorm uses a single Rsqrt activation (batched for m=1..7) instead
#    of Ln+Exp so the activation table loads twice total; pass2 alternates
#    Vector/GpSimd.
import sys
import os as _os

for _p in ("/opt/trn_rl_repo",):
    if _p not in sys.path:
        sys.path.insert(0, _p)

import numpy as np
import ml_dtypes

import concourse.bass as bass
import concourse.bacc as bacc
import concourse.mybir as mybir
import concourse.tile as tile
from concourse import bass_utils

F32 = mybir.dt.float32
BF16 = mybir.dt.bfloat16
F8 = mybir.dt.float8e4
AF = mybir.ActivationFunctionType
ALU = mybir.AluOpType
DR = mybir.MatmulPerfMode.DoubleRow

B, S, D, H, HD = 2, 4096, 768, 12, 64
W, G = 256, 128
EPS = 1e-5
SCALE = 1.0 / np.sqrt(HD)

T = 1024                 # query rows per core
HALO = T + 2 * W         # 1536 banded kv rows per core
KT = G + HALO            # 1664 total kv rows (128 global + halo)
NBT = 12                 # band kcol tiles
NKP = KT // 128          # 13 v partition tiles
ONES = 1.0 / 64.0        # denominator column value (bakes the fp8 x64 scale)

# fp8 weight quantization scales, indexed like the host wstack:
# 0:q(pre-scaled) 1:k 2:v 3:kg 4:vg 5:qg(pre-scaled) 6:o
WSC = [512.0, 64.0, 64.0, 64.0, 64.0, 512.0, 64.0]

NQ = [128, 256, 384, 512, 640, 640, 640, 640, 512, 384, 256, 128]
MOFF = [0]
for _n in NQ:
    MOFF.append(MOFF[-1] + _n)
BAND_COLS = MOFF[-1]     # 5120
PTG0 = BAND_COLS         # ptg occupies cols [5120, 6144)
HCOLS = PTG0 + T         # 6144 score cols per head


def _lo(t):
    return max(0, t - 4)


# score segments: (t, gstart, width, qstart), split at the 512 psum-bank grid.
SEGS = []
for _t in range(13):
    if _t < 12:
        _g0, _nq, _l = MOFF[_t], NQ[_t], _lo(_t)
    else:
        _g0, _nq, _l = PTG0, T, 0
    _s = _g0
    while _s < _g0 + _nq:
        _e = min(_g0 + _nq, (_s // 512 + 1) * 512)
        SEGS.append((_t, _s, _e - _s, 128 * _l + (_s - _g0)))
        _s = _e
CHUNKS = [[sg for sg in SEGS if sg[1] // 1024 == c] for c in range(6)]

# PV out-column segments per band tile: (t, qc_start, width, rhs_off)
PVSEGS = []
for _t in range(12):
    _q0, _q1 = 128 * _lo(_t), 128 * _lo(_t) + NQ[_t]
    _s = _q0
    while _s < _q1:
        _e = min(_q1, (_s // 512 + 1) * 512)
        PVSEGS.append((_t, _s, _e - _s, MOFF[_t] + (_s - _q0)))
        _s = _e


def _emit(tc, dt):
    nc = tc.nc

    with (
        tc.tile_pool(name="const", bufs=1) as constp,
        tc.tile_pool(name="pers", bufs=1) as pers,
    ):
        # ---- ACT table warmup: Exp first (Rsqrt loads once at the tail) ----
        warm = constp.tile([1, 16], F32, tag="warm", name="warm")
        nc.vector.memset(warm[:], 1.0)
        nc.scalar.activation(warm[:], warm[:], AF.Exp)

        # ---- constants ----
        biasT = constp.tile([128, 24], F32, tag="biasT", name="biasT")
        nc.sync.dma_start(biasT[:], dt["biasT"][:])
        bias_t = {}
        for row, name in ((0, "bq"), (1, "bk"), (2, "bkg"), (3, "bqg")):
            bias_t[name] = [biasT[:, row * 6 + p:row * 6 + p + 1]
                            for p in range(6)]
        msel = constp.tile([128, 2], F32, tag="msel", name="msel")
        nc.sync.dma_start(msel[:], dt["msel"][:])
        vrow = {}
        for row, name in ((0, "bv"), (1, "bvg"), (2, "gam"), (3, "bet")):
            t_ = constp.tile([128, D], BF16, tag=name, name=name)
            nc.sync.dma_start(t_[:], dt["vrep"][row])
            vrow[name] = t_
        maskm = constp.tile([128, BAND_COLS], BF16, tag="maskm", name="maskm")

        # ---- persistent activation storage ----
        kT = [pers.tile([128, KT], BF16, tag=f"kT{p}", name=f"kT{p}") for p in range(6)]
        qT = [pers.tile([128, T], BF16, tag=f"qT{p}", name=f"qT{p}") for p in range(6)]
        kgfT = [pers.tile([128, T], BF16, tag=f"kgfT{p}", name=f"kgfT{p}") for p in range(6)]
        qgT = [pers.tile([128, G], BF16, tag=f"qgT{p}", name=f"qgT{p}") for p in range(6)]
        vsb = [pers.tile([128, H * (HD + 1)], BF16, tag=f"v{p}", name=f"v{p}") for p in range(NKP)]
        vgf = [pers.tile([128, H * (HD + 1)], BF16, tag=f"vg{p}", name=f"vg{p}") for p in range(8)]
        # fp8 context (x64), paired along D for the DoubleRow output Dense
        ctx3 = [pers.tile([128, 2 * T], F8, tag=f"ctx{j}", name=f"ctx{j}") for j in range(3)]
        # bf16 (x64) windowed/folded global-row context for the m=0 tile
        ctxGw = [pers.tile([128, G], BF16, tag=f"cgw{p}", name=f"cgw{p}") for p in range(6)]
        ctxG = [pers.tile([128, G], BF16, tag=f"cg{p}", name=f"cg{p}") for p in range(6)]
        ogsb = pers.tile([128, H * G], BF16, tag="ogsb", name="ogsb")
        ogred = pers.tile([128, H * G], BF16, tag="ogred", name="ogred")
        ogd_cm = tc.tile_pool(name="ogdram", bufs=1, space="DRAM")
        ogd = ogd_cm.__enter__()
        og_in = ogd.tile([65, H * G], BF16, tag="og_in", name="og_in")
        og_out = ogd.tile([65, H * G], BF16, tag="og_out", name="og_out")

        nc.gpsimd.memset(ogsb[:], 0.0)
        for p in range(NKP):
            nc.gpsimd.memset(vsb[p][:], ONES)
        for p in range(8):
            nc.gpsimd.memset(vgf[p][:], ONES)

        with tc.tile_pool(name="ptp", bufs=2) as ptp:
            pts = {}

            def pt_tile():
                return ptp.tile([128, 2 * HCOLS], BF16, tag="pt", name="pt")

            with (
                tc.tile_pool(name="xw", bufs=2) as xw,
                tc.tile_pool(name="psA", bufs=2, space="PSUM") as psA,
                tc.tile_pool(name="ptgp", bufs=2) as ptgp,
                tc.tile_pool(name="xtp", bufs=1) as xtp,
            ):
                def x_tile(nm):
                    t_ = xtp.tile([128, 6 * KT], F8, tag=nm, name=nm)
                    nc.sync.dma_start(t_[:], dt[nm][:])
                    return t_[:].rearrange("p (j i c) -> p j i c", i=2, c=KT)

                def load_w(widx):
                    t_ = xw.tile([128, 6 * D], F8, tag="w8t", name="w8t")
                    nc.sync.dma_start(t_[:], dt["w8"][widx])
                    return t_[:].rearrange("p (j i c) -> p j i c", i=2, c=D)

                wk = load_w(1)
                xav = x_tile("xa")
                xbv = x_tile("xb")
                wq = load_w(0)
                nc.sync.dma_start(maskm[:], dt["maskm"][:])
                XAB = None  # set below once both pumps exist

                def projT_group(wt, xts, out_tiles, bias, desc, xcol0, m,
                                nn, gw):
                    ps = psA.tile([128, 1024], F32, tag="pa", name="pa")
                    steps = [(j, xt) for xt in xts for j in range(3)]
                    for si, (j, xt) in enumerate(steps):
                        p0 = 0
                        while p0 < gw:
                            pw = min(512, gw - p0)
                            nc.tensor.matmul(
                                ps[:, p0:p0 + pw],
                                wt[:, j, :, 128 * m:128 * m + 128],
                                xt[:, j, :,
                                   xcol0 + nn + p0:xcol0 + nn + p0 + pw],
                                start=(si == 0), stop=(si == len(steps) - 1),
                                perf_mode=DR)
                            p0 += pw
                    nc.vector.tensor_scalar(
                        out_tiles[m][:, nn:nn + gw], ps[:, :gw],
                        desc, bias[m][:], ALU.mult, ALU.add)

                def proj_T(wt, xts, out_tiles, bias, desc, xcol0, ncols):
                    for m in range(6):
                        nn = 0
                        while nn < ncols:
                            gw = min(1024, ncols - nn)
                            projT_group(wt, xts, out_tiles, bias, desc,
                                        xcol0, m, nn, gw)
                            nn += gw

                def projN_group(wt, xts, out_tiles, brow, desc, xcol0, m):
                    ps = psA.tile([128, 1024], F32, tag="pa", name="pa")
                    steps = [(j, xt) for xt in xts for j in range(3)]
                    for si, (j, xt) in enumerate(steps):
                        for p0, pw in ((0, 512), (512, 256)):
                            nc.tensor.matmul(
                                ps[:, p0:p0 + pw],
                                xt[:, j, :,
                                   xcol0 + 128 * m:xcol0 + 128 * m + 128],
                                wt[:, j, :, p0:p0 + pw],
                                start=(si == 0), stop=(si == len(steps) - 1),
                                perf_mode=DR)
                    ov = out_tiles[m][:].rearrange("p (h e) -> p h e",
                                                   e=HD + 1)
                    for n0, nw in ((0, 512), (512, 256)):
                        h0 = n0 // HD
                        nh = nw // HD
                        nc.vector.scalar_tensor_tensor(
                            ov[:, h0:h0 + nh, :HD],
                            ps[:, n0:n0 + nw].rearrange("p (h e) -> p h e",
                                                        e=HD),
                            desc,
                            vrow[brow][:, n0:n0 + nw]
                            .rearrange("p (h e) -> p h e", e=HD),
                            ALU.mult, ALU.add)

                def qk_chunk(pr, pt, c, pool):
                    pss = [pool.tile([128, 1024], F32, tag="pa", name="pa")
                           for _ in (0, 1)]
                    for (t, gs, w_, qs) in CHUNKS[c]:
                        for half in (0, 1):
                            r0, r1 = 64 * half, 64 * half + 64
                            lt = (kT[pr][r0:r1, 0:G] if t == 12 else
                                  kT[pr][r0:r1, G + 128 * t:G + 128 * t + 128])
                            nc.tensor.matmul(
                                pss[half][:, gs - 1024 * c:gs - 1024 * c + w_],
                                lt, qT[pr][r0:r1, qs:qs + w_],
                                start=True, stop=True, skip_group_check=True)
                    mw = min(BAND_COLS - 1024 * c, 1024)  # mask cols in chunk
                    for half in (0, 1):
                        b = HCOLS * half + 1024 * c
                        nc.scalar.activation(
                            pt[:, b:b + 1024], pss[half][:], AF.Exp)
                        if mw > 0:
                            nc.vector.tensor_tensor(
                                pt[:, b:b + mw], pt[:, b:b + mw],
                                maskm[:, 1024 * c:1024 * c + mw], ALU.mult)

                def ptog_pr(pr, ptgsb):
                    for grp in (0, 1):
                        ps = psA.tile([128, 1024], F32, tag="pa", name="pa")
                        for tt in range(4 * grp, 4 * grp + 4):
                            for half in (0, 1):
                                r0, r1 = 64 * half, 64 * half + 64
                                nc.tensor.matmul(
                                    ps[:, 512 * half + 128 * (tt - 4 * grp):
                                       512 * half + 128 * (tt - 4 * grp) + 128],
                                    kgfT[pr][r0:r1, 128 * tt:128 * tt + 128],
                                    qgT[pr][r0:r1, :],
                                    start=True, stop=True,
                                    skip_group_check=True)
                        nc.scalar.activation(
                            ptgsb[:, 1024 * grp:1024 * grp + 1024],
                            ps[:], AF.Exp)

                def ognum_head(pr, half, ptgsb):
                    h = 2 * pr + half
                    ps = psA.tile([128, 1024], F32, tag="pa", name="pa")
                    for tt in range(8):
                        nc.tensor.matmul(
                            ps[:65, :G],
                            vgf[tt][:, (HD + 1) * h:(HD + 1) * h + HD + 1],
                            ptgsb[:, 1024 * (tt // 4) + 512 * half
                                  + 128 * (tt % 4):
                                  1024 * (tt // 4) + 512 * half
                                  + 128 * (tt % 4) + 128],
                            start=(tt == 0), stop=(tt == 7))
                    nc.vector.tensor_copy(ogsb[:65, G * h:G * h + G],
                                          ps[:65, :G])

                # ---- emission: projections pipelined with pairs 0/1 ----
                XAB = [xav, xbv]
                XA = [xav]
                proj_T(wk, XAB, kT, bias_t["bk"], 1.0 / WSC[1], 0, KT)
                proj_T(wq, XAB, qT, bias_t["bq"], 1.0 / WSC[0], G + W, T)
                wv = load_w(2)
                for m in range(NKP):
                    projN_group(wv, XA, vsb, "bv", 1.0 / WSC[2], 0, m)

                # pair0 QK chunks interleaved with kgf/qg/vgf projections
                pts[0] = pt_tile()
                wkg = load_w(3)
                wqg = load_w(5)
                wvg = load_w(4)
                projq = []
                for m in range(6):
                    projq.append(("T", wkg, XAB, kgfT, bias_t["bkg"],
                                  1.0 / WSC[3], G + W, m, 0, 1024))
                for m in range(6):
                    projq.append(("T", wqg, XAB, qgT, bias_t["bqg"],
                                  1.0 / WSC[5], 0, m, 0, G))
                for m in range(8):
                    projq.append(("N", wvg, XA, vgf, "bvg",
                                  1.0 / WSC[4], G + W, m))
                ci = 0
                for i, job in enumerate(projq):
                    if job[0] == "T":
                        _, wt, xts, ot, bi, de, xc, m, nn, gw = job
                        projT_group(wt, xts, ot, bi, de, xc, m, nn, gw)
                    else:
                        _, wt, xts, ot, br, de, xc, m = job
                        projN_group(wt, xts, ot, br, de, xc, m)
                    if i % 2 == 1 and ci < 6:
                        qk_chunk(0, pts[0], ci, psA)
                        ci += 1
                while ci < 6:
                    qk_chunk(0, pts[0], ci, psA)
                    ci += 1

                # ptog + og-num per pr, interleaved with pair1 QK chunks
                pts[1] = pt_tile()
                ci = 0
                for pr in range(6):
                    ptgsb = ptgp.tile([128, 2048], BF16, tag="ptgsb",
                                      name="ptgsb")
                    ptog_pr(pr, ptgsb)
                    ognum_head(pr, 0, ptgsb)
                    ognum_head(pr, 1, ptgsb)
                    if ci < 6:
                        qk_chunk(1, pts[1], ci, psA)
                        ci += 1
                while ci < 6:
                    qk_chunk(1, pts[1], ci, psA)
                    ci += 1

                # ship the partials and kick the AllReduce as soon as the
                # numerators land; consumers are emitted dead last.
                nc.sync.dma_start(og_in[:], ogsb[:65, :])
                if _os.environ.get("NO_CC") == "1":
                    nc.sync.dma_start(og_out[:], og_in[:])
                else:
                    nc.gpsimd.collective_compute(
                        "AllReduce", ALU.add,
                        replica_groups=[[0, 1, 2, 3], [4, 5, 6, 7]],
                        ins=[og_in.opt()], outs=[og_out.opt()])
                nc.sync.dma_start(ogred[:65, :], og_out[:])

            # ---- band: PV(pr) interleaved with QK(pr+2) ----
            with (
                tc.tile_pool(name="psB", bufs=2, space="PSUM") as psB,
                tc.tile_pool(name="psC", bufs=2, space="PSUM") as psC,
                tc.tile_pool(name="nrm", bufs=2) as nrm,
            ):
                def pv_batch(pr, half, pt, ps, seglist):
                    b = HCOLS * half
                    h = 2 * pr + half
                    vcol = slice((HD + 1) * h, (HD + 1) * h + HD + 1)
                    for kind, qs, w_, ro, t in seglist:
                        if kind == "g":
                            nc.tensor.matmul(
                                ps[:, qs:qs + w_], vsb[0][:, vcol],
                                pt[:, b + PTG0 + ro:b + PTG0 + ro + w_],
                                start=True, stop=False)
                        else:
                            isstop = (t == 7 and qs == 384) or t == 11
                            nc.tensor.matmul(
                                ps[:, qs:qs + w_], vsb[1 + t][:, vcol],
                                pt[:, b + ro:b + ro + w_],
                                start=False, stop=isstop,
                                skip_group_check=not isstop)

                # PV work for one head: global cols first (start), then bands
                PVLIST = ([("g", 0, 512, 0, -1), ("g", 512, 512, 512, -1)]
                          + [("b", qs, w_, ro, t)
                             for (t, qs, w_, ro) in PVSEGS])

                def norm_head(pr, half, ps):
                    # denominator (already /64): stage to SBUF, reciprocal
                    dst = nrm.tile([1, T], F32, tag=f"dst{half}",
                                   name=f"dst{half}", bufs=1)
                    nc.vector.tensor_copy(dst[:], ps[64:65, :])
                    dinv = nrm.tile([1, T], F32, tag=f"dinv{half}",
                                    name=f"dinv{half}", bufs=1)
                    nc.vector.reciprocal_approx_fast(dinv[:], dst[:])
                    invb = nrm.tile([HD, T], F32, tag=f"invb{half}",
                                    name=f"invb{half}", bufs=1)
                    nc.gpsimd.partition_broadcast(invb[:], dinv[0:1, :])
                    r0 = 64 * half
                    kk = pr % 2
                    nc.vector.tensor_tensor(
                        ctx3[pr // 2][r0:r0 + HD, kk * T:kk * T + T],
                        ps[:HD, :], invb[:], ALU.mult)
                    nc.vector.tensor_tensor(
                        ctxGw[pr][r0:r0 + HD, :],
                        ps[:HD, :G], invb[:, :G], ALU.mult)

                for pr in range(6):
                    # QK of pair pr+2 chunks interleave with PV of pair pr
                    qkc = list(range(6)) if pr + 2 <= 5 else []
                    if qkc:
                        pts[pr + 2] = pt_tile()
                    psh = []
                    for half in (0, 1):
                        ps = psB.tile([65, 1024], F32, tag="pv", name="pv")
                        psh.append(ps)
                    # split PVLIST into 3 batches; alternate h0/h1 per seg
                    nb = 3
                    bsz = (len(PVLIST) + nb - 1) // nb
                    for j in range(nb):
                        if qkc and j < len(qkc):
                            qk_chunk(pr + 2, pts[pr + 2], qkc[j], psC)
                        for seg in PVLIST[j * bsz:(j + 1) * bsz]:
                            for half in (0, 1):
                                pv_batch(pr, half, pts[pr], psh[half], [seg])
                    for j in range(nb, 6):
                        if qkc and j < len(qkc):
                            qk_chunk(pr + 2, pts[pr + 2], qkc[j], psC)
                    for half in (0, 1):
                        norm_head(pr, half, psh[half])

        # ---- output Dense + residual + LayerNorm ----
        with (
            tc.tile_pool(name="wo", bufs=1) as wop,
            tc.tile_pool(name="ln", bufs=2) as lnp,
            tc.tile_pool(name="psD", bufs=2, space="PSUM") as psD,
        ):
            wo8t = wop.tile([128, 6 * D], F8, tag="wo8", name="wo8")
            nc.sync.dma_start(wo8t[:], dt["w8"][6])
            wov = wo8t[:].rearrange("p (j i c) -> p j i c", i=2, c=D)
            wob = []
            for k in range(6):
                t_ = wop.tile([128, D], BF16, tag=f"wob{k}", name=f"wob{k}")
                nc.sync.dma_start(t_[:], dt["wob"][128 * k:128 * k + 128, :])
                wob.append(t_)
            epst = wop.tile([128, 1], F32, tag="epst", name="epst")
            nc.gpsimd.memset(epst[:], EPS)
            sumsq = wop.tile([128, 8], F32, tag="sumsq", name="sumsq")
            istd = wop.tile([128, 8], F32, tag="istd", name="istd")
            ycs = [wop.tile([128, D], F32, tag=f"yc{m}", name=f"yc{m}")
                   for m in range(8)]
            c3v = [ctx3[j][:].rearrange("p (k t) -> p k t", t=T)
                   for j in range(3)]

            def ln_stats(m, ys, rs, ps, desc):
                sums = lnp.tile([128, 2], F32, tag="sums", name="sums")
                nc.vector.scalar_tensor_tensor(
                    ys[:], ps[:, :D], desc, rs[:], ALU.mult, ALU.add,
                    accum_out=sums[:, 0:1])
                negmean = lnp.tile([128, 1], F32, tag="negmean",
                                   name="negmean")
                nc.vector.tensor_scalar_mul(negmean[:], sums[:, 0:1],
                                            -1.0 / D)
                nc.vector.tensor_scalar(ycs[m][:], ys[:], negmean[:], None,
                                        ALU.add)
                nc.vector.scalar_tensor_tensor(ys[:], ycs[m][:], 1.0,
                                               ycs[m][:], ALU.mult, ALU.mult,
                                               accum_out=sumsq[:, m:m + 1])

            def ln_pass1(m):
                ys = lnp.tile([128, D], F32, tag="ys", name="ys")
                rs = lnp.tile([128, D], F32, tag="rs", name="rs")
                nc.sync.dma_start(rs[:], dt["res"][128 * m:128 * m + 128, :])
                ps = psD.tile([128, 1024], F32, tag="pd", name="pd")
                for n0, nw in ((0, 512), (512, 256)):
                    for j in range(3):
                        nc.tensor.matmul(
                            ps[:, n0:n0 + nw],
                            c3v[j][:, :, 128 * m:128 * m + 128],
                            wov[:, j, :, n0:n0 + nw],
                            start=(j == 0), stop=(j == 2), perf_mode=DR)
                ln_stats(m, ys, rs, ps, 1.0 / (WSC[6] * 64.0))

            def ln_pass2(m, eng):
                yo = lnp.tile([128, D], F32, tag="yo", name="yo")
                eng.scalar_tensor_tensor(
                    yo[:], ycs[m][:], istd[:, m:m + 1], vrow["gam"][:],
                    ALU.mult, ALU.mult)
                eng.tensor_tensor(yo[:], yo[:], vrow["bet"][:], ALU.add)
                nc.sync.dma_start(dt["y"][128 * m:128 * m + 128, :], yo[:])

            sstd = wop.tile([128, 8], F32, tag="sstd", name="sstd")

            for m in range(1, 8):
                ln_pass1(m)
            nc.scalar.activation(sstd[:, 1:8], sumsq[:, 1:8], AF.Sqrt,
                                 bias=epst[:], scale=1.0 / D)
            nc.vector.reciprocal_approx_fast(istd[:, 1:8], sstd[:, 1:8])
            for m in range(1, 8):
                ln_pass2(m, nc.vector)

            # ---- fold the AllReduced global-attention output (last) ----
            dstg = wop.tile([1, H * G], F32, tag="dstg", name="dstg")
            nc.vector.tensor_copy(dstg[:], ogred[64:65, :])
            dinvg = wop.tile([1, H * G], F32, tag="dinvg", name="dinvg")
            nc.vector.reciprocal_approx_fast(dinvg[:], dstg[:])
            dinvm = wop.tile([1, H * G], F32, tag="dinvm", name="dinvm")
            nc.vector.tensor_scalar_mul(dinvm[:], dinvg[:], msel[0:1, 0:1])
            finvb = wop.tile([64, H * G], F32, tag="finvb", name="finvb")
            nc.gpsimd.partition_broadcast(finvb[:], dinvm[0:1, :])
            for hh in range(12):
                pr, half = hh // 2, hh % 2
                r0 = 64 * half
                t1 = lnp.tile([128, G], F32, tag="ft1", name="ft1")
                nc.gpsimd.tensor_tensor(
                    t1[r0:r0 + 64, :], ogred[:64, G * hh:G * hh + G],
                    finvb[:, G * hh:G * hh + G], ALU.mult)
                nc.vector.tensor_scalar_mul(
                    ctxG[pr][r0:r0 + 64, :], ctxGw[pr][r0:r0 + 64, :],
                    msel[r0:r0 + 64, 1:2])
                nc.gpsimd.tensor_tensor(
                    ctxG[pr][r0:r0 + 64, :], ctxG[pr][r0:r0 + 64, :],
                    t1[r0:r0 + 64, :], ALU.add)

            # ---- m=0 tile: bf16 Dense off ctxG, then its LayerNorm ----
            ys0 = lnp.tile([128, D], F32, tag="ys", name="ys")
            rs0 = lnp.tile([128, D], F32, tag="rs", name="rs")
            nc.sync.dma_start(rs0[:], dt["res"][0:128, :])
            ps0 = psD.tile([128, 1024], F32, tag="pd", name="pd")
            for n0, nw in ((0, 512), (512, 256)):
                for k in range(6):
                    nc.tensor.matmul(
                        ps0[:, n0:n0 + nw], ctxG[k][:, :],
                        wob[k][:, n0:n0 + nw],
                        start=(k == 0), stop=(k == 5))
            ln_stats(0, ys0, rs0, ps0, 1.0 / 64.0)
            nc.scalar.activation(sstd[:, 0:1], sumsq[:, 0:1], AF.Sqrt,
                                 bias=epst[:], scale=1.0 / D)
            nc.vector.reciprocal_approx_fast(istd[:, 0:1], sstd[:, 0:1])
            ln_pass2(0, nc.vector)
            if "dbg_ctx" in dt:
                for j in range(3):
                    nc.sync.dma_start(dt["dbg_ctx"][j], ctx3[j][:])
                nc.sync.dma_start(dt["dbg_og"][:], ogred[:65, :])
        ogd_cm.__exit__(None, None, None)


def build_nc():
    nc = bacc.Bacc(trn_type="TRN2", num_devices=8)
    dt = {
        "xa": nc.dram_tensor("xa", [128, 6 * KT], F8, kind="ExternalInput"),
        "xb": nc.dram_tensor("xb", [128, 6 * KT], F8, kind="ExternalInput"),
        "w8": nc.dram_tensor("w8", [7, 128, 6 * D], F8, kind="ExternalInput"),
        "wob": nc.dram_tensor("wob", [D, D], BF16, kind="ExternalInput"),
        "res": nc.dram_tensor("res", [T, D], F32, kind="ExternalInput"),
        "maskm": nc.dram_tensor("maskm", [128, BAND_COLS], BF16,
                                kind="ExternalInput"),
        "msel": nc.dram_tensor("msel", [128, 2], F32, kind="ExternalInput"),
        "vrep": nc.dram_tensor("vrep", [4, 128, D], BF16, kind="ExternalInput"),
        "biasT": nc.dram_tensor("biasT", [128, 24], F32, kind="ExternalInput"),
        "y": nc.dram_tensor("y", [T, D], F32, kind="ExternalOutput"),
    }
    if _os.environ.get("DBG") == "1":
        dt["dbg_ctx"] = nc.dram_tensor("dbg_ctx", [3, 128, 2 * T], F8,
                                       kind="ExternalOutput")
        dt["dbg_og"] = nc.dram_tensor("dbg_og", [65, H * G], BF16,
                                      kind="ExternalOutput")
    with tile.TileContext(nc) as tc:
        _emit(tc, dt)
    nc.compile()
    return nc


def host_inputs(inputs):
    """Build the 8 per-core input maps from the full problem inputs."""
    hs = np.asarray(inputs["hidden_states"], np.float32)
    assert hs.shape == (B, S, D)
    E4 = ml_dtypes.float8_e4m3
    bf = lambda a: np.ascontiguousarray(np.asarray(a, np.float32)).astype(
        ml_dtypes.bfloat16)
    f32 = lambda a: np.ascontiguousarray(np.asarray(a, np.float32))

    def pair8(a2d, ncols):
        # [768, ncols] fp8 -> [128, 3, 2, ncols] -> [128, 6*ncols]
        return np.ascontiguousarray(
            a2d.reshape(3, 2, 128, ncols).transpose(2, 0, 1, 3)
            .reshape(128, 6 * ncols))

    wstack = np.stack([
        np.asarray(inputs["Wq"], np.float32) * SCALE,
        np.asarray(inputs["Wk"], np.float32),
        np.asarray(inputs["Wv"], np.float32),
        np.asarray(inputs["Wkg"], np.float32),
        np.asarray(inputs["Wvg"], np.float32),
        np.asarray(inputs["Wqg"], np.float32) * SCALE,
        np.asarray(inputs["Wo"], np.float32),
    ])
    w8 = np.stack([
        pair8((wstack[i] * WSC[i]).astype(E4), D) for i in range(7)])
    wob = bf(wstack[6])

    vecs = np.stack([
        np.asarray(inputs["bq"], np.float32) * SCALE,
        np.asarray(inputs["bk"], np.float32),
        np.asarray(inputs["bkg"], np.float32),
        np.asarray(inputs["bqg"], np.float32) * SCALE,
    ])
    bo = np.asarray(inputs["bo"], np.float32)
    biasT = np.zeros((128, 24), np.float32)
    for row in range(4):
        for p in range(6):
            biasT[:, row * 6 + p] = vecs[row, 128 * p:128 * p + 128]
    vrep = bf(np.broadcast_to(
        np.stack([
            np.asarray(inputs["bv"], np.float32),
            np.asarray(inputs["bvg"], np.float32),
            np.asarray(inputs["ln_gamma"], np.float32),
            np.asarray(inputs["ln_beta"], np.float32),
        ])[:, None, :], (4, 128, D)))

    in_maps = []
    for c in range(8):
        b, j = c // 4, c % 4
        r0 = j * T
        x = hs[b]
        xp = np.zeros((S + 2 * W, D), np.float32)
        xp[W:W + S] = x
        x_kv = np.concatenate([x[:G], xp[r0:r0 + HALO]], axis=0)  # [1664, D]
        xT = np.ascontiguousarray(x_kv.T)                          # [768, 1664]
        xa8 = xT.astype(E4)
        xb8 = (xT - xa8.astype(np.float32)).astype(E4)
        res = f32(x[r0:r0 + T] + bo)

        # multiplicative band mask, {0,1} bf16, MOFF layout
        mask = np.zeros((128, BAND_COLS), np.float32)
        for t in range(NBT):
            lo = _lo(t)
            nq = NQ[t]
            jj = np.arange(128 * t, 128 * t + 128)[:, None]
            ii = np.arange(lo * 128, lo * 128 + nq)[None, :]
            kpos = r0 - W + jj
            valid = ((jj - ii >= 0) & (jj - ii <= 2 * W)
                     & (kpos >= G) & (kpos < S))
            mask[:, MOFF[t]:MOFF[t] + nq] = valid.astype(np.float32)

        m = 1.0 if j == 0 else 0.0
        msel = np.zeros((128, 2), np.float32)
        msel[:, 0] = m
        msel[:, 1] = 1.0 - m

        in_maps.append({
            "xa": pair8(xa8, KT), "xb": pair8(xb8, KT),
            "w8": w8, "wob": wob, "res": res,
            "maskm": bf(mask), "msel": f32(msel),
            "vrep": vrep, "biasT": biasT,
        })
    return in_maps


_NC_CACHE = {}


def _get_nc():
    if "nc" not in _NC_CACHE:
        _NC_CACHE["nc"] = build_nc()
    return _NC_CACHE["nc"]


def kernel(**inputs) -> np.ndarray:
    # sanity-check the fixed global-attention pattern this kernel hardcodes
    iga = np.asarray(inputs["is_index_global_attn"])
    assert iga.shape == (B, S)
    expect = np.broadcast_to(np.arange(S) < G, (B, S))
    assert np.array_equal(iga, expect), "kernel hardcodes a G=128 prefix"
    am = np.asarray(inputs["attention_mask"], np.float32)
    assert np.all(am == 0.0), "kernel assumes no key-padding mask"

    nc = _get_nc()
    in_maps = host_inputs(inputs)
    res = bass_utils.run_bass_kernel_spmd(nc, in_maps, core_ids=list(range(8)))
    outs = res.results if hasattr(res, "results") else res
    y = np.zeros((B, S, D), np.float32)
    for c in range(8):
        b, j = c // 4, c % 4
        y[b, j * T:(j + 1) * T] = outs[c]["y"]
    return y


if __name__ == "__main__":
    nc = build_nc()
    print("build ok; instructions:",
          sum(len(bb.instructions) for bb in nc.main_func.blocks))


# revision 20
# speedup vs baseline: 1.1613x; 1.0102x over previous
# Trainium2 Bass kernel for nn_LongformerSelfAttentionPegasus (B=2,S=4096,D=768,
# H=12,HD=64, window W=256 one-sided, G=128 global prefix tokens).
#
# Sharding (8 NeuronCores): sequence-parallel — core c handles batch c//4,
# query rows [1024*(c%4), 1024*(c%4+1)). Banded attention is fully local (the
# host ships a +/-W halo of the hidden states). The global-query attention
# (rows 0..G attend to all S tokens through the *_global projections) is
# token-parallel: each core computes exp-score partials (numerator+denominator
# via a 1/64-column on V) over its own 1024 tokens, and a [65, H*G] bf16
# AllReduce within each 4-core batch group completes the softmax. The final
# Dense + residual + LayerNorm are row-local, so no further communication.
#
# Perf structure (v2):
#  - All seven projections run in fp8e4m3 with the DoubleRow perf mode
#    (K=256 per matmul at 0.5 cycles/col). Weights are quantized per-tensor
#    (x64, x512 for the pre-scaled q/qg) which costs <1e-3 rel err; the
#    activations are double-pumped (x = xa + xb residual split) on the
#    q/k/kg/qg paths where score error is exp-amplified, single-pumped on
#    v/vg. The context is written as fp8 (x64) so the output Dense also runs
#    DoubleRow.
#  - Band masking is a multiplicative {0,1} bf16 mask applied on VectorE
#    after exp; two heads share a partition tile and are emitted as adjacent
#    K=64 matmuls at base partitions 0/64 so they run concurrently in
#    separate PE row groups; PV accumulates wide-N with a global-column
#    start=True pass; softmax denominators ride a 1/64 ones-column so the
#    reciprocal (read straight out of PSUM) is already the x64 fp8 scale.
#  - The AllReduce is emitted before the band loop (trigger on GpSimd as
#    soon as the numerators' DMA lands) and its consumers are isolated: the
#    folded global rows live in separate ctxG tiles feeding only the m=0
#    output-Dense tile, which is emitted dead last together with the fold,
#    so a slow collective can never stall the band pipeline or the other
#    7/8 of the Dense+LayerNorm.
#  - LayerNorm uses a single Rsqrt activation (batched for m=1..7) instead
#    of Ln+Exp so the activation table loads twice total; pass2 alternates
#    Vector/GpSimd.
import sys
import os as _os

for _p in ("/opt/trn_rl_repo",):
    if _p not in sys.path:
        sys.path.insert(0, _p)

import numpy as np
import ml_dtypes

import concourse.bass as bass
import concourse.bacc as bacc
import concourse.mybir as mybir
import concourse.tile as tile
from concourse import bass_utils

F32 = mybir.dt.float32
BF16 = mybir.dt.bfloat16
F8 = mybir.dt.float8e4
AF = mybir.ActivationFunctionType
ALU = mybir.AluOpType
DR = mybir.MatmulPerfMode.DoubleRow

B, S, D, H, HD = 2, 4096, 768, 12, 64
W, G = 256, 128
EPS = 1e-5
SCALE = 1.0 / np.sqrt(HD)

T = 1024                 # query rows per core
HALO = T + 2 * W         # 1536 banded kv rows per core
KT = G + HALO            # 1664 total kv rows (128 global + halo)
NBT = 12                 # band kcol tiles
NKP = KT // 128          # 13 v partition tiles
ONES = 1.0 / 64.0        # denominator column value (bakes the fp8 x64 scale)

# fp8 weight quantization scales, indexed like the host wstack:
# 0:q(pre-scaled) 1:k 2:v 3:kg 4:vg 5:qg(pre-scaled) 6:o
WSC = [512.0, 64.0, 64.0, 64.0, 64.0, 512.0, 64.0]

NQ = [128, 256, 384, 512, 640, 640, 640, 640, 512, 384, 256, 128]
MOFF = [0]
for _n in NQ:
    MOFF.append(MOFF[-1] + _n)
BAND_COLS = MOFF[-1]     # 5120
PTG0 = BAND_COLS         # ptg occupies cols [5120, 6144)
HCOLS = PTG0 + T         # 6144 score cols per head


def _lo(t):
    return max(0, t - 4)


# score segments: (t, gstart, width, qstart), split at the 512 psum-bank grid.
SEGS = []
for _t in range(13):
    if _t < 12:
        _g0, _nq, _l = MOFF[_t], NQ[_t], _lo(_t)
    else:
        _g0, _nq, _l = PTG0, T, 0
    _s = _g0
    while _s < _g0 + _nq:
        _e = min(_g0 + _nq, (_s // 512 + 1) * 512)
        SEGS.append((_t, _s, _e - _s, 128 * _l + (_s - _g0)))
        _s = _e
CHUNKS = [[sg for sg in SEGS if sg[1] // 1024 == c] for c in range(6)]

# PV out-column segments per band tile: (t, qc_start, width, rhs_off)
PVSEGS = []
for _t in range(12):
    _q0, _q1 = 128 * _lo(_t), 128 * _lo(_t) + NQ[_t]
    _s = _q0
    while _s < _q1:
        _e = min(_q1, (_s // 512 + 1) * 512)
        PVSEGS.append((_t, _s, _e - _s, MOFF[_t] + (_s - _q0)))
        _s = _e


def _emit(tc, dt):
    nc = tc.nc

    with (
        tc.tile_pool(name="const", bufs=1) as constp,
        tc.tile_pool(name="pers", bufs=1) as pers,
    ):
        # ---- ACT table warmup: Exp first (Rsqrt loads once at the tail) ----
        warm = constp.tile([1, 16], F32, tag="warm", name="warm")
        nc.vector.memset(warm[:], 1.0)
        nc.scalar.activation(warm[:], warm[:], AF.Exp)

        # ---- constants ----
        biasT = constp.tile([128, 24], F32, tag="biasT", name="biasT")
        nc.sync.dma_start(biasT[:], dt["biasT"][:])
        bias_t = {}
        for row, name in ((0, "bq"), (1, "bk"), (2, "bkg"), (3, "bqg")):
            bias_t[name] = [biasT[:, row * 6 + p:row * 6 + p + 1]
                            for p in range(6)]
        msel = constp.tile([128, 2], F32, tag="msel", name="msel")
        nc.sync.dma_start(msel[:], dt["msel"][:])
        vrow = {}
        for row, name in ((0, "bv"), (1, "bvg"), (2, "gam"), (3, "bet")):
            vrow[name] = constp.tile([128, D], BF16, tag=name, name=name)
        maskm = constp.tile([128, BAND_COLS], BF16, tag="maskm", name="maskm")
        # bf16 residual (x + bo), all 8 row-tiles in one early DMA
        rs8 = constp.tile([128, 8 * D], BF16, tag="rs8", name="rs8")

        # ---- persistent activation storage ----
        kT = [pers.tile([128, KT], BF16, tag=f"kT{p}", name=f"kT{p}") for p in range(6)]
        qT = [pers.tile([128, T], BF16, tag=f"qT{p}", name=f"qT{p}") for p in range(6)]
        kgfT = [pers.tile([128, T], BF16, tag=f"kgfT{p}", name=f"kgfT{p}") for p in range(6)]
        qgT = [pers.tile([128, G], BF16, tag=f"qgT{p}", name=f"qgT{p}") for p in range(6)]
        vsb = [pers.tile([128, H * (HD + 1)], BF16, tag=f"v{p}", name=f"v{p}") for p in range(NKP)]
        vgf = [pers.tile([128, H * (HD + 1)], BF16, tag=f"vg{p}", name=f"vg{p}") for p in range(8)]
        # fp8 context (x64), paired along D for the DoubleRow output Dense
        ctx3 = [pers.tile([128, 2 * T], F8, tag=f"ctx{j}", name=f"ctx{j}") for j in range(3)]
        # bf16 (x64) windowed/folded global-row context for the m=0 tile
        ctxGw = [pers.tile([128, G], BF16, tag=f"cgw{p}", name=f"cgw{p}") for p in range(6)]
        ctxG = [pers.tile([128, G], BF16, tag=f"cg{p}", name=f"cg{p}") for p in range(6)]
        ogsb = pers.tile([128, H * G], BF16, tag="ogsb", name="ogsb")
        ogred = pers.tile([128, H * G], BF16, tag="ogred", name="ogred")
        ogd_cm = tc.tile_pool(name="ogdram", bufs=1, space="DRAM")
        ogd = ogd_cm.__enter__()
        og_in = ogd.tile([65, H * G], BF16, tag="og_in", name="og_in")
        og_out = ogd.tile([65, H * G], BF16, tag="og_out", name="og_out")

        nc.gpsimd.memset(ogsb[:], 0.0)
        for p in range(NKP):
            nc.gpsimd.memset(vsb[p][:], ONES)
        for p in range(8):
            nc.gpsimd.memset(vgf[p][:], ONES)

        with tc.tile_pool(name="ptp", bufs=2) as ptp:
            pts = {}

            def pt_tile():
                return ptp.tile([128, 2 * HCOLS], BF16, tag="pt", name="pt")

            with (
                tc.tile_pool(name="xw", bufs=2) as xw,
                tc.tile_pool(name="psA", bufs=2, space="PSUM") as psA,
                tc.tile_pool(name="ptgp", bufs=2) as ptgp,
                tc.tile_pool(name="xtp", bufs=1) as xtp,
            ):
                def x_tile(nm):
                    t_ = xtp.tile([128, 6 * KT], F8, tag=nm, name=nm)
                    nc.sync.dma_start(t_[:], dt[nm][:])
                    return t_[:].rearrange("p (j i c) -> p j i c", i=2, c=KT)

                def load_w(widx):
                    t_ = xw.tile([128, 6 * D], F8, tag="w8t", name="w8t")
                    nc.sync.dma_start(t_[:], dt["w8"][widx])
                    return t_[:].rearrange("p (j i c) -> p j i c", i=2, c=D)

                wk = load_w(1)
                xav = x_tile("xa")
                wq = load_w(0)
                xbv = x_tile("xb")
                wv = load_w(2)
                for row, name in ((0, "bv"), (1, "bvg"), (2, "gam"),
                                  (3, "bet")):
                    nc.sync.dma_start(vrow[name][:], dt["vrep"][row])
                nc.sync.dma_start(maskm[:], dt["maskm"][:])
                nc.sync.dma_start(
                    rs8[:].rearrange("p (m c) -> p m c", c=D),
                    dt["res"][:].rearrange("(m p) c -> p m c", p=128))

                def projT_group(wt, xts, out_tiles, bias, desc, xcol0, m,
                                nn, gw):
                    ps = psA.tile([128, 1024], F32, tag="pa", name="pa")
                    steps = [(j, xt) for xt in xts for j in range(3)]
                    for si, (j, xt) in enumerate(steps):
                        p0 = 0
                        while p0 < gw:
                            pw = min(512, gw - p0)
                            nc.tensor.matmul(
                                ps[:, p0:p0 + pw],
                                wt[:, j, :, 128 * m:128 * m + 128],
                                xt[:, j, :,
                                   xcol0 + nn + p0:xcol0 + nn + p0 + pw],
                                start=(si == 0), stop=(si == len(steps) - 1),
                                perf_mode=DR)
                            p0 += pw
                    nc.vector.tensor_scalar(
                        out_tiles[m][:, nn:nn + gw], ps[:, :gw],
                        desc, bias[m][:], ALU.mult, ALU.add)

                def proj_T(wt, xts, out_tiles, bias, desc, xcol0, ncols):
                    for m in range(6):
                        nn = 0
                        while nn < ncols:
                            gw = min(1024, ncols - nn)
                            projT_group(wt, xts, out_tiles, bias, desc,
                                        xcol0, m, nn, gw)
                            nn += gw

                def projN_group(wt, xts, out_tiles, brow, desc, xcol0, m):
                    ps = psA.tile([128, 1024], F32, tag="pa", name="pa")
                    steps = [(j, xt) for xt in xts for j in range(3)]
                    for si, (j, xt) in enumerate(steps):
                        for p0, pw in ((0, 512), (512, 256)):
                            nc.tensor.matmul(
                                ps[:, p0:p0 + pw],
                                xt[:, j, :,
                                   xcol0 + 128 * m:xcol0 + 128 * m + 128],
                                wt[:, j, :, p0:p0 + pw],
                                start=(si == 0), stop=(si == len(steps) - 1),
                                perf_mode=DR)
                    ov = out_tiles[m][:].rearrange("p (h e) -> p h e",
                                                   e=HD + 1)
                    for n0, nw in ((0, 512), (512, 256)):
                        h0 = n0 // HD
                        nh = nw // HD
                        nc.vector.scalar_tensor_tensor(
                            ov[:, h0:h0 + nh, :HD],
                            ps[:, n0:n0 + nw].rearrange("p (h e) -> p h e",
                                                        e=HD),
                            desc,
                            vrow[brow][:, n0:n0 + nw]
                            .rearrange("p (h e) -> p h e", e=HD),
                            ALU.mult, ALU.add)

                def qk_chunk(pr, pt, c, pool):
                    pss = [pool.tile([128, 1024], F32, tag="pa", name="pa")
                           for _ in (0, 1)]
                    for (t, gs, w_, qs) in CHUNKS[c]:
                        for half in (0, 1):
                            r0, r1 = 64 * half, 64 * half + 64
                            lt = (kT[pr][r0:r1, 0:G] if t == 12 else
                                  kT[pr][r0:r1, G + 128 * t:G + 128 * t + 128])
                            nc.tensor.matmul(
                                pss[half][:, gs - 1024 * c:gs - 1024 * c + w_],
                                lt, qT[pr][r0:r1, qs:qs + w_],
                                start=True, stop=True, skip_group_check=True)
                    mw = min(BAND_COLS - 1024 * c, 1024)  # mask cols in chunk
                    for half in (0, 1):
                        b = HCOLS * half + 1024 * c
                        nc.scalar.activation(
                            pt[:, b:b + 1024], pss[half][:], AF.Exp)
                        if mw > 0:
                            nc.vector.tensor_tensor(
                                pt[:, b:b + mw], pt[:, b:b + mw],
                                maskm[:, 1024 * c:1024 * c + mw], ALU.mult)

                def ptog_pr(pr, ptgsb):
                    for grp in (0, 1):
                        ps = psA.tile([128, 1024], F32, tag="pa", name="pa")
                        for tt in range(4 * grp, 4 * grp + 4):
                            for half in (0, 1):
                                r0, r1 = 64 * half, 64 * half + 64
                                nc.tensor.matmul(
                                    ps[:, 512 * half + 128 * (tt - 4 * grp):
                                       512 * half + 128 * (tt - 4 * grp) + 128],
                                    kgfT[pr][r0:r1, 128 * tt:128 * tt + 128],
                                    qgT[pr][r0:r1, :],
                                    start=True, stop=True,
                                    skip_group_check=True)
                        nc.scalar.activation(
                            ptgsb[:, 1024 * grp:1024 * grp + 1024],
                            ps[:], AF.Exp)

                def ognum_head(pr, half, ptgsb):
                    h = 2 * pr + half
                    ps = psA.tile([128, 1024], F32, tag="pa", name="pa")
                    for tt in range(8):
                        nc.tensor.matmul(
                            ps[:65, :G],
                            vgf[tt][:, (HD + 1) * h:(HD + 1) * h + HD + 1],
                            ptgsb[:, 1024 * (tt // 4) + 512 * half
                                  + 128 * (tt % 4):
                                  1024 * (tt // 4) + 512 * half
                                  + 128 * (tt % 4) + 128],
                            start=(tt == 0), stop=(tt == 7))
                    nc.vector.tensor_copy(ogsb[:65, G * h:G * h + G],
                                          ps[:65, :G])

                # ---- emission: projections pipelined with pairs 0/1 ----
                XAB = [xav, xbv]
                XA = [xav]
                proj_T(wk, XAB, kT, bias_t["bk"], 1.0 / WSC[1], 0, KT)
                proj_T(wq, XAB, qT, bias_t["bq"], 1.0 / WSC[0], G + W, T)
                for m in range(NKP):
                    projN_group(wv, XA, vsb, "bv", 1.0 / WSC[2], 0, m)

                # pair0 QK chunks interleaved with kgf/qg/vgf projections
                pts[0] = pt_tile()
                wkg = load_w(3)
                wqg = load_w(5)
                wvg = load_w(4)
                projq = []
                for m in range(6):
                    projq.append(("T", wkg, XAB, kgfT, bias_t["bkg"],
                                  1.0 / WSC[3], G + W, m, 0, 1024))
                for m in range(6):
                    projq.append(("T", wqg, XAB, qgT, bias_t["bqg"],
                                  1.0 / WSC[5], 0, m, 0, G))
                for m in range(8):
                    projq.append(("N", wvg, XA, vgf, "bvg",
                                  1.0 / WSC[4], G + W, m))
                ci = 0
                for i, job in enumerate(projq):
                    if job[0] == "T":
                        _, wt, xts, ot, bi, de, xc, m, nn, gw = job
                        projT_group(wt, xts, ot, bi, de, xc, m, nn, gw)
                    else:
                        _, wt, xts, ot, br, de, xc, m = job
                        projN_group(wt, xts, ot, br, de, xc, m)
                    if i % 2 == 1 and ci < 6:
                        qk_chunk(0, pts[0], ci, psA)
                        ci += 1
                while ci < 6:
                    qk_chunk(0, pts[0], ci, psA)
                    ci += 1

                # ptog + og-num per pr, interleaved with pair1 QK chunks
                pts[1] = pt_tile()
                ci = 0
                for pr in range(6):
                    ptgsb = ptgp.tile([128, 2048], BF16, tag="ptgsb",
                                      name="ptgsb")
                    ptog_pr(pr, ptgsb)
                    ognum_head(pr, 0, ptgsb)
                    ognum_head(pr, 1, ptgsb)
                    if ci < 6:
                        qk_chunk(1, pts[1], ci, psA)
                        ci += 1
                while ci < 6:
                    qk_chunk(1, pts[1], ci, psA)
                    ci += 1

                # ship the partials and kick the AllReduce as soon as the
                # numerators land; consumers are emitted dead last.
                nc.sync.dma_start(og_in[:], ogsb[:65, :])
                if _os.environ.get("NO_CC") == "1":
                    nc.sync.dma_start(og_out[:], og_in[:])
                else:
                    nc.gpsimd.collective_compute(
                        "AllReduce", ALU.add,
                        replica_groups=[[0, 1, 2, 3], [4, 5, 6, 7]],
                        ins=[og_in.opt()], outs=[og_out.opt()])
                nc.sync.dma_start(ogred[:65, :], og_out[:])

            # ---- band: PV(pr) interleaved with QK(pr+2) ----
            with (
                tc.tile_pool(name="psB", bufs=2, space="PSUM") as psB,
                tc.tile_pool(name="psC", bufs=2, space="PSUM") as psC,
                tc.tile_pool(name="nrm", bufs=2) as nrm,
            ):
                def pv_batch(pr, half, pt, ps, seglist):
                    b = HCOLS * half
                    h = 2 * pr + half
                    vcol = slice((HD + 1) * h, (HD + 1) * h + HD + 1)
                    for kind, qs, w_, ro, t in seglist:
                        if kind == "g":
                            nc.tensor.matmul(
                                ps[:, qs:qs + w_], vsb[0][:, vcol],
                                pt[:, b + PTG0 + ro:b + PTG0 + ro + w_],
                                start=True, stop=False)
                        else:
                            isstop = (t == 7 and qs == 384) or t == 11
                            nc.tensor.matmul(
                                ps[:, qs:qs + w_], vsb[1 + t][:, vcol],
                                pt[:, b + ro:b + ro + w_],
                                start=False, stop=isstop,
                                skip_group_check=not isstop)

                # PV work for one head: global cols first (start), then bands
                PVLIST = ([("g", 0, 512, 0, -1), ("g", 512, 512, 512, -1)]
                          + [("b", qs, w_, ro, t)
                             for (t, qs, w_, ro) in PVSEGS])

                def norm_head(pr, half, ps):
                    # denominator (already /64): stage to SBUF, reciprocal
                    dst = nrm.tile([1, T], F32, tag=f"dst{half}",
                                   name=f"dst{half}", bufs=1)
                    nc.vector.tensor_copy(dst[:], ps[64:65, :])
                    dinv = nrm.tile([1, T], F32, tag=f"dinv{half}",
                                    name=f"dinv{half}", bufs=1)
                    nc.vector.reciprocal_approx_fast(dinv[:], dst[:])
                    invb = nrm.tile([HD, T], F32, tag=f"invb{half}",
                                    name=f"invb{half}", bufs=1)
                    nc.gpsimd.partition_broadcast(invb[:], dinv[0:1, :])
                    r0 = 64 * half
                    kk = pr % 2
                    nc.vector.tensor_tensor(
                        ctx3[pr // 2][r0:r0 + HD, kk * T:kk * T + T],
                        ps[:HD, :], invb[:], ALU.mult)
                    nc.vector.tensor_tensor(
                        ctxGw[pr][r0:r0 + HD, :],
                        ps[:HD, :G], invb[:, :G], ALU.mult)

                for pr in range(6):
                    # QK of pair pr+2 chunks interleave with PV of pair pr
                    qkc = list(range(6)) if pr + 2 <= 5 else []
                    if qkc:
                        pts[pr + 2] = pt_tile()
                    psh = []
                    for half in (0, 1):
                        ps = psB.tile([65, 1024], F32, tag="pv", name="pv")
                        psh.append(ps)
                    # split PVLIST into 3 batches; alternate h0/h1 per seg
                    nb = 3
                    bsz = (len(PVLIST) + nb - 1) // nb
                    for j in range(nb):
                        if qkc and j < len(qkc):
                            qk_chunk(pr + 2, pts[pr + 2], qkc[j], psC)
                        for seg in PVLIST[j * bsz:(j + 1) * bsz]:
                            for half in (0, 1):
                                pv_batch(pr, half, pts[pr], psh[half], [seg])
                    for j in range(nb, 6):
                        if qkc and j < len(qkc):
                            qk_chunk(pr + 2, pts[pr + 2], qkc[j], psC)
                    for half in (0, 1):
                        norm_head(pr, half, psh[half])

        # ---- output Dense + residual + LayerNorm ----
        with (
            tc.tile_pool(name="wo", bufs=1) as wop,
            tc.tile_pool(name="ln", bufs=2) as lnp,
            tc.tile_pool(name="psD", bufs=2, space="PSUM") as psD,
        ):
            wo8t = wop.tile([128, 6 * D], F8, tag="wo8", name="wo8")
            nc.sync.dma_start(wo8t[:], dt["w8"][6])
            wov = wo8t[:].rearrange("p (j i c) -> p j i c", i=2, c=D)
            wob = []
            for k in range(6):
                t_ = wop.tile([128, D], BF16, tag=f"wob{k}", name=f"wob{k}")
                nc.sync.dma_start(t_[:], dt["wob"][128 * k:128 * k + 128, :])
                wob.append(t_)
            epst = wop.tile([128, 1], F32, tag="epst", name="epst")
            nc.gpsimd.memset(epst[:], EPS)
            sumsq = wop.tile([128, 8], F32, tag="sumsq", name="sumsq")
            istd = wop.tile([128, 8], F32, tag="istd", name="istd")
            ycs = [wop.tile([128, D], F32, tag=f"yc{m}", name=f"yc{m}")
                   for m in range(8)]
            c3v = [ctx3[j][:].rearrange("p (k t) -> p k t", t=T)
                   for j in range(3)]

            sstd = wop.tile([128, 8], F32, tag="sstd", name="sstd")
            rs8v = rs8[:].rearrange("p (m c) -> p m c", c=D)

            def ln_stats(m, ys, ps, desc):
                sums = lnp.tile([128, 2], F32, tag="sums", name="sums")
                nc.vector.scalar_tensor_tensor(
                    ys[:], ps[:, :D], desc, rs8v[:, m, :], ALU.mult, ALU.add,
                    accum_out=sums[:, 0:1])
                negmean = lnp.tile([128, 1], F32, tag="negmean",
                                   name="negmean")
                nc.vector.tensor_scalar_mul(negmean[:], sums[:, 0:1],
                                            -1.0 / D)
                nc.vector.tensor_scalar(ycs[m][:], ys[:], negmean[:], None,
                                        ALU.add)
                nc.vector.scalar_tensor_tensor(ys[:], ycs[m][:], 1.0,
                                               ycs[m][:], ALU.mult, ALU.mult,
                                               accum_out=sumsq[:, m:m + 1])

            def ln_finalize(m):
                nc.scalar.activation(sstd[:, m:m + 1], sumsq[:, m:m + 1],
                                     AF.Sqrt, bias=epst[:], scale=1.0 / D)
                nc.vector.reciprocal_approx_fast(istd[:, m:m + 1],
                                                 sstd[:, m:m + 1])
                yo = lnp.tile([128, D], F32, tag="yo", name="yo")
                nc.vector.scalar_tensor_tensor(
                    yo[:], ycs[m][:], istd[:, m:m + 1], vrow["gam"][:],
                    ALU.mult, ALU.mult)
                nc.gpsimd.tensor_tensor(yo[:], yo[:], vrow["bet"][:], ALU.add)
                nc.sync.dma_start(dt["y"][128 * m:128 * m + 128, :], yo[:])

            def ln_pass1(m):
                ys = lnp.tile([128, D], F32, tag="ys", name="ys")
                ps = psD.tile([128, 1024], F32, tag="pd", name="pd")
                for n0, nw in ((0, 512), (512, 256)):
                    for j in range(3):
                        nc.tensor.matmul(
                            ps[:, n0:n0 + nw],
                            c3v[j][:, :, 128 * m:128 * m + 128],
                            wov[:, j, :, n0:n0 + nw],
                            start=(j == 0), stop=(j == 2), perf_mode=DR)
                ln_stats(m, ys, ps, 1.0 / (WSC[6] * 64.0))

            for m in range(1, 8):
                ln_pass1(m)
                ln_finalize(m)

            # ---- fold the AllReduced global-attention output (last) ----
            dstg = wop.tile([1, H * G], F32, tag="dstg", name="dstg")
            nc.vector.tensor_copy(dstg[:], ogred[64:65, :])
            dinvg = wop.tile([1, H * G], F32, tag="dinvg", name="dinvg")
            nc.vector.reciprocal_approx_fast(dinvg[:], dstg[:])
            dinvm = wop.tile([1, H * G], F32, tag="dinvm", name="dinvm")
            nc.vector.tensor_scalar_mul(dinvm[:], dinvg[:], msel[0:1, 0:1])
            finvb = wop.tile([64, H * G], F32, tag="finvb", name="finvb")
            nc.gpsimd.partition_broadcast(finvb[:], dinvm[0:1, :])
            for hh in range(12):
                pr, half = hh // 2, hh % 2
                r0 = 64 * half
                t1 = lnp.tile([128, G], F32, tag="ft1", name="ft1")
                nc.gpsimd.tensor_tensor(
                    t1[r0:r0 + 64, :], ogred[:64, G * hh:G * hh + G],
                    finvb[:, G * hh:G * hh + G], ALU.mult)
                nc.vector.tensor_scalar_mul(
                    ctxG[pr][r0:r0 + 64, :], ctxGw[pr][r0:r0 + 64, :],
                    msel[r0:r0 + 64, 1:2])
                nc.gpsimd.tensor_tensor(
                    ctxG[pr][r0:r0 + 64, :], ctxG[pr][r0:r0 + 64, :],
                    t1[r0:r0 + 64, :], ALU.add)

            # ---- m=0 tile: bf16 Dense off ctxG, then its LayerNorm ----
            ys0 = lnp.tile([128, D], F32, tag="ys", name="ys")
            ps0 = psD.tile([128, 1024], F32, tag="pd", name="pd")
            for n0, nw in ((0, 512), (512, 256)):
                for k in range(6):
                    nc.tensor.matmul(
                        ps0[:, n0:n0 + nw], ctxG[k][:, :],
                        wob[k][:, n0:n0 + nw],
                        start=(k == 0), stop=(k == 5))
            ln_stats(0, ys0, ps0, 1.0 / 64.0)
            ln_finalize(0)
            if "dbg_ctx" in dt:
                for j in range(3):
                    nc.sync.dma_start(dt["dbg_ctx"][j], ctx3[j][:])
                nc.sync.dma_start(dt["dbg_og"][:], ogred[:65, :])
        ogd_cm.__exit__(None, None, None)


def build_nc():
    nc = bacc.Bacc(trn_type="TRN2", num_devices=8)
    dt = {
        "xa": nc.dram_tensor("xa", [128, 6 * KT], F8, kind="ExternalInput"),
        "xb": nc.dram_tensor("xb", [128, 6 * KT], F8, kind="ExternalInput"),
        "w8": nc.dram_tensor("w8", [7, 128, 6 * D], F8, kind="ExternalInput"),
        "wob": nc.dram_tensor("wob", [D, D], BF16, kind="ExternalInput"),
        "res": nc.dram_tensor("res", [T, D], BF16, kind="ExternalInput"),
        "maskm": nc.dram_tensor("maskm", [128, BAND_COLS], BF16,
                                kind="ExternalInput"),
        "msel": nc.dram_tensor("msel", [128, 2], F32, kind="ExternalInput"),
        "vrep": nc.dram_tensor("vrep", [4, 128, D], BF16, kind="ExternalInput"),
        "biasT": nc.dram_tensor("biasT", [128, 24], F32, kind="ExternalInput"),
        "y": nc.dram_tensor("y", [T, D], F32, kind="ExternalOutput"),
    }
    if _os.environ.get("DBG") == "1":
        dt["dbg_ctx"] = nc.dram_tensor("dbg_ctx", [3, 128, 2 * T], F8,
                                       kind="ExternalOutput")
        dt["dbg_og"] = nc.dram_tensor("dbg_og", [65, H * G], BF16,
                                      kind="ExternalOutput")
    with tile.TileContext(nc) as tc:
        _emit(tc, dt)
    nc.compile()
    return nc


def host_inputs(inputs):
    """Build the 8 per-core input maps from the full problem inputs."""
    hs = np.asarray(inputs["hidden_states"], np.float32)
    assert hs.shape == (B, S, D)
    E4 = ml_dtypes.float8_e4m3
    bf = lambda a: np.ascontiguousarray(np.asarray(a, np.float32)).astype(
        ml_dtypes.bfloat16)
    f32 = lambda a: np.ascontiguousarray(np.asarray(a, np.float32))

    def pair8(a2d, ncols):
        # [768, ncols] fp8 -> [128, 3, 2, ncols] -> [128, 6*ncols]
        return np.ascontiguousarray(
            a2d.reshape(3, 2, 128, ncols).transpose(2, 0, 1, 3)
            .reshape(128, 6 * ncols))

    wstack = np.stack([
        np.asarray(inputs["Wq"], np.float32) * SCALE,
        np.asarray(inputs["Wk"], np.float32),
        np.asarray(inputs["Wv"], np.float32),
        np.asarray(inputs["Wkg"], np.float32),
        np.asarray(inputs["Wvg"], np.float32),
        np.asarray(inputs["Wqg"], np.float32) * SCALE,
        np.asarray(inputs["Wo"], np.float32),
    ])
    w8 = np.stack([
        pair8((wstack[i] * WSC[i]).astype(E4), D) for i in range(7)])
    wob = bf(wstack[6])

    vecs = np.stack([
        np.asarray(inputs["bq"], np.float32) * SCALE,
        np.asarray(inputs["bk"], np.float32),
        np.asarray(inputs["bkg"], np.float32),
        np.asarray(inputs["bqg"], np.float32) * SCALE,
    ])
    bo = np.asarray(inputs["bo"], np.float32)
    biasT = np.zeros((128, 24), np.float32)
    for row in range(4):
        for p in range(6):
            biasT[:, row * 6 + p] = vecs[row, 128 * p:128 * p + 128]
    vrep = bf(np.broadcast_to(
        np.stack([
            np.asarray(inputs["bv"], np.float32),
            np.asarray(inputs["bvg"], np.float32),
            np.asarray(inputs["ln_gamma"], np.float32),
            np.asarray(inputs["ln_beta"], np.float32),
        ])[:, None, :], (4, 128, D)))

    in_maps = []
    for c in range(8):
        b, j = c // 4, c % 4
        r0 = j * T
        x = hs[b]
        xp = np.zeros((S + 2 * W, D), np.float32)
        xp[W:W + S] = x
        x_kv = np.concatenate([x[:G], xp[r0:r0 + HALO]], axis=0)  # [1664, D]
        xT = np.ascontiguousarray(x_kv.T)                          # [768, 1664]
        xa8 = xT.astype(E4)
        xb8 = (xT - xa8.astype(np.float32)).astype(E4)
        res = bf(x[r0:r0 + T] + bo)

        # multiplicative band mask, {0,1} bf16, MOFF layout
        mask = np.zeros((128, BAND_COLS), np.float32)
        for t in range(NBT):
            lo = _lo(t)
            nq = NQ[t]
            jj = np.arange(128 * t, 128 * t + 128)[:, None]
            ii = np.arange(lo * 128, lo * 128 + nq)[None, :]
            kpos = r0 - W + jj
            valid = ((jj - ii >= 0) & (jj - ii <= 2 * W)
                     & (kpos >= G) & (kpos < S))
            mask[:, MOFF[t]:MOFF[t] + nq] = valid.astype(np.float32)

        m = 1.0 if j == 0 else 0.0
        msel = np.zeros((128, 2), np.float32)
        msel[:, 0] = m
        msel[:, 1] = 1.0 - m

        in_maps.append({
            "xa": pair8(xa8, KT), "xb": pair8(xb8, KT),
            "w8": w8, "wob": wob, "res": res,
            "maskm": bf(mask), "msel": f32(msel),
            "vrep": vrep, "biasT": biasT,
        })
    return in_maps


_NC_CACHE = {}


def _get_nc():
    if "nc" not in _NC_CACHE:
        _NC_CACHE["nc"] = build_nc()
    return _NC_CACHE["nc"]


def kernel(**inputs) -> np.ndarray:
    # sanity-check the fixed global-attention pattern this kernel hardcodes
    iga = np.asarray(inputs["is_index_global_attn"])
    assert iga.shape == (B, S)
    expect = np.broadcast_to(np.arange(S) < G, (B, S))
    assert np.array_equal(iga, expect), "kernel hardcodes a G=128 prefix"
    am = np.asarray(inputs["attention_mask"], np.float32)
    assert np.all(am == 0.0), "kernel assumes no key-padding mask"

    nc = _get_nc()
    in_maps = host_inputs(inputs)
    res = bass_utils.run_bass_kernel_spmd(nc, in_maps, core_ids=list(range(8)))
    outs = res.results if hasattr(res, "results") else res
    y = np.zeros((B, S, D), np.float32)
    for c in range(8):
        b, j = c // 4, c % 4
        y[b, j * T:(j + 1) * T] = outs[c]["y"]
    return y


if __name__ == "__main__":
    nc = build_nc()
    print("build ok; instructions:",
          sum(len(bb.instructions) for bb in nc.main_func.blocks))


# revision 38
# speedup vs baseline: 1.2068x; 1.0392x over previous
# Trainium2 Bass kernel for nn_LongformerSelfAttentionPegasus (B=2,S=4096,D=768,
# H=12,HD=64, window W=256 one-sided, G=128 global prefix tokens).
#
# Sharding (8 NeuronCores): sequence-parallel — core c handles batch c//4,
# query rows [1024*(c%4), 1024*(c%4+1)). Banded attention is fully local (the
# host ships a +/-W halo of the hidden states). The global-query attention
# (rows 0..G attend to all S tokens through the *_global projections) is
# token-parallel: each core computes exp-score partials (numerator+denominator
# via a 1/64-column on V) over its own 1024 tokens, and a [65, H*G] bf16
# AllReduce within each 4-core batch group completes the softmax. The final
# Dense + residual + LayerNorm are row-local, so no further communication.
#
# Perf structure (v2):
#  - All seven projections run in fp8e4m3 with the DoubleRow perf mode
#    (K=256 per matmul at 0.5 cycles/col). Weights are quantized per-tensor
#    (x64, x512 for the pre-scaled q/qg) which costs <1e-3 rel err; the
#    activations are double-pumped (x = xa + xb residual split) on the
#    q/k/kg/qg paths where score error is exp-amplified, single-pumped on
#    v/vg. The context is written as fp8 (x64) so the output Dense also runs
#    DoubleRow.
#  - Band masking is a multiplicative {0,1} bf16 mask applied on VectorE
#    after exp; two heads share a partition tile and are emitted as adjacent
#    K=64 matmuls at base partitions 0/64 so they run concurrently in
#    separate PE row groups; PV accumulates wide-N with a global-column
#    start=True pass; softmax denominators ride a 1/64 ones-column so the
#    reciprocal (read straight out of PSUM) is already the x64 fp8 scale.
#  - The AllReduce is emitted before the band loop (trigger on GpSimd as
#    soon as the numerators' DMA lands) and its consumers are isolated: the
#    folded global rows live in separate ctxG tiles feeding only the m=0
#    output-Dense tile, which is emitted dead last together with the fold,
#    so a slow collective can never stall the band pipeline or the other
#    7/8 of the Dense+LayerNorm.
#  - LayerNorm uses a single Rsqrt activation (batched for m=1..7) instead
#    of Ln+Exp so the activation table loads twice total; pass2 alternates
#    Vector/GpSimd.
import sys
import os as _os

for _p in ("/opt/trn_rl_repo",):
    if _p not in sys.path:
        sys.path.insert(0, _p)

import numpy as np
import ml_dtypes

import concourse.bass as bass
import concourse.bacc as bacc
import concourse.mybir as mybir
import concourse.tile as tile
from concourse import bass_utils

F32 = mybir.dt.float32
BF16 = mybir.dt.bfloat16
F8 = mybir.dt.float8e4
AF = mybir.ActivationFunctionType
ALU = mybir.AluOpType
DR = mybir.MatmulPerfMode.DoubleRow

B, S, D, H, HD = 2, 4096, 768, 12, 64
W, G = 256, 128
EPS = 1e-5
SCALE = 1.0 / np.sqrt(HD)

T = 1024                 # query rows per core
HALO = T + 2 * W         # 1536 banded kv rows per core
KT = G + HALO            # 1664 total kv rows (128 global + halo)
NBT = 12                 # band kcol tiles
NKP = KT // 128          # 13 v partition tiles
ONES = 1.0 / 64.0        # denominator column value (bakes the fp8 x64 scale)

# fp8 weight quantization scales, indexed like the host wstack:
# 0:q(pre-scaled) 1:k 2:v 3:kg 4:vg 5:qg(pre-scaled) 6:o
WSC = [512.0, 64.0, 64.0, 64.0, 64.0, 512.0, 64.0]

NQ = [128, 256, 384, 512, 640, 640, 640, 640, 512, 384, 256, 128]
MOFF = [0]
for _n in NQ:
    MOFF.append(MOFF[-1] + _n)
BAND_COLS = MOFF[-1]     # 5120
PTG0 = BAND_COLS         # ptg occupies cols [5120, 6144)
HCOLS = PTG0 + T         # 6144 score cols per head


def _lo(t):
    return max(0, t - 4)


# score segments: (t, gstart, width, qstart), split at the 512 psum-bank grid.
SEGS = []
for _t in range(13):
    if _t < 12:
        _g0, _nq, _l = MOFF[_t], NQ[_t], _lo(_t)
    else:
        _g0, _nq, _l = PTG0, T, 0
    _s = _g0
    while _s < _g0 + _nq:
        _e = min(_g0 + _nq, (_s // 512 + 1) * 512)
        SEGS.append((_t, _s, _e - _s, 128 * _l + (_s - _g0)))
        _s = _e
CHUNKS = [[sg for sg in SEGS if sg[1] // 1024 == c] for c in range(6)]

# PV out-column segments per band tile: (t, qc_start, width, rhs_off)
PVSEGS = []
for _t in range(12):
    _q0, _q1 = 128 * _lo(_t), 128 * _lo(_t) + NQ[_t]
    _s = _q0
    while _s < _q1:
        _e = min(_q1, (_s // 512 + 1) * 512)
        PVSEGS.append((_t, _s, _e - _s, MOFF[_t] + (_s - _q0)))
        _s = _e


def _emit(tc, dt):
    nc = tc.nc

    with (
        tc.tile_pool(name="const", bufs=1) as constp,
        tc.tile_pool(name="pers", bufs=1) as pers,
    ):
        # ---- ACT table warmup: Exp first (Rsqrt loads once at the tail) ----
        warm = constp.tile([1, 16], F32, tag="warm", name="warm")
        nc.vector.memset(warm[:], 1.0)
        nc.scalar.activation(warm[:], warm[:], AF.Exp)

        # ---- constants ----
        biasT = constp.tile([128, 24], F32, tag="biasT", name="biasT")
        nc.sync.dma_start(biasT[:], dt["biasT"][:])
        bias_t = {}
        for row, name in ((0, "bq"), (1, "bk"), (2, "bkg"), (3, "bqg")):
            bias_t[name] = [biasT[:, row * 6 + p:row * 6 + p + 1]
                            for p in range(6)]
        msel = constp.tile([128, 2], F32, tag="msel", name="msel")
        nc.sync.dma_start(msel[:], dt["msel"][:])
        vrow = {}
        for row, name in ((0, "bv"), (1, "bvg"), (2, "gam"), (3, "bet")):
            vrow[name] = constp.tile([128, D], BF16, tag=name, name=name)
        maskm = constp.tile([128, BAND_COLS], BF16, tag="maskm", name="maskm")
        # bf16 residual (x + bo), all 8 row-tiles in one early DMA
        rs8 = constp.tile([128, 8 * D], BF16, tag="rs8", name="rs8")

        # ---- persistent activation storage ----
        kT = [pers.tile([128, KT], BF16, tag=f"kT{p}", name=f"kT{p}") for p in range(6)]
        qT = [pers.tile([128, T], BF16, tag=f"qT{p}", name=f"qT{p}") for p in range(6)]
        kgfT = [pers.tile([128, T], BF16, tag=f"kgfT{p}", name=f"kgfT{p}") for p in range(6)]
        qgT = [pers.tile([128, G], BF16, tag=f"qgT{p}", name=f"qgT{p}") for p in range(6)]
        vsb = [pers.tile([128, H * (HD + 1)], BF16, tag=f"v{p}", name=f"v{p}") for p in range(NKP)]
        vgf = [pers.tile([128, H * (HD + 1)], BF16, tag=f"vg{p}", name=f"vg{p}") for p in range(8)]
        # fp8 context (x64), paired along D for the DoubleRow output Dense
        ctx3 = [pers.tile([128, 2 * T], F8, tag=f"ctx{j}", name=f"ctx{j}") for j in range(3)]
        # bf16 (x64) windowed/folded global-row context for the m=0 tile
        ctxGw = [pers.tile([128, G], BF16, tag=f"cgw{p}", name=f"cgw{p}") for p in range(6)]
        ctxG = [pers.tile([128, G], BF16, tag=f"cg{p}", name=f"cg{p}") for p in range(6)]
        ogsb = pers.tile([128, H * G], BF16, tag="ogsb", name="ogsb")
        ogred = pers.tile([128, H * G], BF16, tag="ogred", name="ogred")
        ogd_cm = tc.tile_pool(name="ogdram", bufs=1, space="DRAM")
        ogd = ogd_cm.__enter__()
        og_in = ogd.tile([65, H * G], BF16, tag="og_in", name="og_in")
        og_out = ogd.tile([65, H * G], BF16, tag="og_out", name="og_out")

        nc.gpsimd.memset(ogsb[:], 0.0)
        for p in range(NKP):
            nc.gpsimd.memset(vsb[p][:], ONES)
        for p in range(8):
            nc.gpsimd.memset(vgf[p][:], ONES)
        # all-ones stationary (any base partition) for PE row-broadcasts
        obc = pers.tile([128, 64], BF16, tag="obc", name="obc")
        nc.gpsimd.memset(obc[:], 1.0)

        with tc.tile_pool(name="ptp", bufs=2) as ptp:
            pts = {}

            def pt_tile():
                return ptp.tile([128, 2 * HCOLS], BF16, tag="pt", name="pt")

            with (
                tc.tile_pool(name="xw", bufs=2) as xw,
                tc.tile_pool(name="psA", bufs=2, space="PSUM") as psA,
                tc.tile_pool(name="ptgp", bufs=2) as ptgp,
                tc.tile_pool(name="xtp", bufs=1) as xtp,
            ):
                def x_tile(nm):
                    t_ = xtp.tile([128, 6 * KT], F8, tag=nm, name=nm)
                    nc.sync.dma_start(t_[:], dt[nm][:])
                    return t_[:].rearrange("p (j i c) -> p j i c", i=2, c=KT)

                def load_w(widx):
                    t_ = xw.tile([128, 6 * D], F8, tag="w8t", name="w8t")
                    nc.sync.dma_start(t_[:], dt["w8"][widx])
                    return t_[:].rearrange("p (j i c) -> p j i c", i=2, c=D)

                wk = load_w(1)
                xav = x_tile("xa")
                wq = load_w(0)
                xbv = x_tile("xb")
                wv = load_w(2)
                for row, name in ((0, "bv"), (1, "bvg"), (2, "gam"),
                                  (3, "bet")):
                    nc.sync.dma_start(vrow[name][:], dt["vrep"][row])
                nc.sync.dma_start(maskm[:], dt["maskm"][:])
                nc.sync.dma_start(
                    rs8[:].rearrange("p (m c) -> p m c", c=D),
                    dt["res"][:].rearrange("(m p) c -> p m c", p=128))

                def projT_group(wt, xts, out_tiles, bias, desc, xcol0, m,
                                nn, gw):
                    ps = psA.tile([128, 1024], F32, tag="pa", name="pa")
                    steps = [(j, xt) for xt in xts for j in range(3)]
                    for si, (j, xt) in enumerate(steps):
                        p0 = 0
                        while p0 < gw:
                            pw = min(512, gw - p0)
                            nc.tensor.matmul(
                                ps[:, p0:p0 + pw],
                                wt[:, j, :, 128 * m:128 * m + 128],
                                xt[:, j, :,
                                   xcol0 + nn + p0:xcol0 + nn + p0 + pw],
                                start=(si == 0), stop=(si == len(steps) - 1),
                                perf_mode=DR)
                            p0 += pw
                    nc.vector.tensor_scalar(
                        out_tiles[m][:, nn:nn + gw], ps[:, :gw],
                        desc, bias[m][:], ALU.mult, ALU.add)

                def proj_T(wt, xts, out_tiles, bias, desc, xcol0, ncols):
                    for m in range(6):
                        nn = 0
                        while nn < ncols:
                            gw = min(1024, ncols - nn)
                            projT_group(wt, xts, out_tiles, bias, desc,
                                        xcol0, m, nn, gw)
                            nn += gw

                def projN_group(wt, xts, out_tiles, brow, desc, xcol0, m):
                    ps = psA.tile([128, 1024], F32, tag="pa", name="pa")
                    steps = [(j, xt) for xt in xts for j in range(3)]
                    for si, (j, xt) in enumerate(steps):
                        for p0, pw in ((0, 512), (512, 256)):
                            nc.tensor.matmul(
                                ps[:, p0:p0 + pw],
                                xt[:, j, :,
                                   xcol0 + 128 * m:xcol0 + 128 * m + 128],
                                wt[:, j, :, p0:p0 + pw],
                                start=(si == 0), stop=(si == len(steps) - 1),
                                perf_mode=DR)
                    ov = out_tiles[m][:, :H * (HD + 1)].rearrange(
                        "p (h e) -> p h e", e=HD + 1)
                    for n0, nw in ((0, 512), (512, 256)):
                        h0 = n0 // HD
                        nh = nw // HD
                        nc.vector.scalar_tensor_tensor(
                            ov[:, h0:h0 + nh, :HD],
                            ps[:, n0:n0 + nw].rearrange("p (h e) -> p h e",
                                                        e=HD),
                            desc,
                            vrow[brow][:, n0:n0 + nw]
                            .rearrange("p (h e) -> p h e", e=HD),
                            ALU.mult, ALU.add)

                def qk_chunk(pr, pt, c, pool):
                    pss = [pool.tile([128, 1024], F32, tag="pa", name="pa")
                           for _ in (0, 1)]
                    for (t, gs, w_, qs) in CHUNKS[c]:
                        for half in (0, 1):
                            r0, r1 = 64 * half, 64 * half + 64
                            lt = (kT[pr][r0:r1, 0:G] if t == 12 else
                                  kT[pr][r0:r1, G + 128 * t:G + 128 * t + 128])
                            nc.tensor.matmul(
                                pss[half][:, gs - 1024 * c:gs - 1024 * c + w_],
                                lt, qT[pr][r0:r1, qs:qs + w_],
                                start=True, stop=True, skip_group_check=True)
                    mw = min(BAND_COLS - 1024 * c, 1024)  # mask cols in chunk
                    for half in (0, 1):
                        b = HCOLS * half + 1024 * c
                        nc.scalar.activation(
                            pt[:, b:b + 1024], pss[half][:], AF.Exp)
                        if mw > 0:
                            nc.vector.tensor_tensor(
                                pt[:, b:b + mw], pt[:, b:b + mw],
                                maskm[:, 1024 * c:1024 * c + mw], ALU.mult)

                def ptog_pr(pr, ptgsb):
                    for grp in (0, 1):
                        ps = psA.tile([128, 1024], F32, tag="pa", name="pa")
                        for tt in range(4 * grp, 4 * grp + 4):
                            for half in (0, 1):
                                r0, r1 = 64 * half, 64 * half + 64
                                nc.tensor.matmul(
                                    ps[:, 512 * half + 128 * (tt - 4 * grp):
                                       512 * half + 128 * (tt - 4 * grp) + 128],
                                    kgfT[pr][r0:r1, 128 * tt:128 * tt + 128],
                                    qgT[pr][r0:r1, :],
                                    start=True, stop=True,
                                    skip_group_check=True)
                        nc.scalar.activation(
                            ptgsb[:, 1024 * grp:1024 * grp + 1024],
                            ps[:], AF.Exp)

                def ognum_head(pr, half, ptgsb):
                    h = 2 * pr + half
                    ps = psA.tile([128, 1024], F32, tag="pa", name="pa")
                    for tt in range(8):
                        nc.tensor.matmul(
                            ps[:65, :G],
                            vgf[tt][:, (HD + 1) * h:(HD + 1) * h + HD + 1],
                            ptgsb[:, 1024 * (tt // 4) + 512 * half
                                  + 128 * (tt % 4):
                                  1024 * (tt // 4) + 512 * half
                                  + 128 * (tt % 4) + 128],
                            start=(tt == 0), stop=(tt == 7))
                    nc.vector.tensor_copy(ogsb[:65, G * h:G * h + G],
                                          ps[:65, :G])

                # ---- emission: projections pipelined with pairs 0/1 ----
                XAB = [xav, xbv]
                XA = [xav]
                proj_T(wk, XAB, kT, bias_t["bk"], 1.0 / WSC[1], 0, KT)
                proj_T(wq, XAB, qT, bias_t["bq"], 1.0 / WSC[0], G + W, T)
                for m in range(NKP):
                    projN_group(wv, XA, vsb, "bv", 1.0 / WSC[2], 0, m)

                # pair0 QK chunks interleaved with kgf/qg/vgf projections
                pts[0] = pt_tile()
                wkg = load_w(3)
                wqg = load_w(5)
                wvg = load_w(4)
                projq = []
                for m in range(6):
                    projq.append(("T", wkg, XAB, kgfT, bias_t["bkg"],
                                  1.0 / WSC[3], G + W, m, 0, 1024))
                for m in range(6):
                    projq.append(("T", wqg, XAB, qgT, bias_t["bqg"],
                                  1.0 / WSC[5], 0, m, 0, G))
                for m in range(8):
                    projq.append(("N", wvg, XA, vgf, "bvg",
                                  1.0 / WSC[4], G + W, m))
                ci = 0
                for i, job in enumerate(projq):
                    if job[0] == "T":
                        _, wt, xts, ot, bi, de, xc, m, nn, gw = job
                        projT_group(wt, xts, ot, bi, de, xc, m, nn, gw)
                    else:
                        _, wt, xts, ot, br, de, xc, m = job
                        projN_group(wt, xts, ot, br, de, xc, m)
                    if i % 2 == 1 and ci < 6:
                        qk_chunk(0, pts[0], ci, psA)
                        ci += 1
                while ci < 6:
                    qk_chunk(0, pts[0], ci, psA)
                    ci += 1

                # ptog + og-num per pr, interleaved with pair1 QK chunks
                pts[1] = pt_tile()
                ci = 0
                for pr in range(6):
                    ptgsb = ptgp.tile([128, 2048], BF16, tag="ptgsb",
                                      name="ptgsb")
                    ptog_pr(pr, ptgsb)
                    ognum_head(pr, 0, ptgsb)
                    ognum_head(pr, 1, ptgsb)
                    if ci < 6:
                        qk_chunk(1, pts[1], ci, psA)
                        ci += 1
                while ci < 6:
                    qk_chunk(1, pts[1], ci, psA)
                    ci += 1

                # ship the partials and kick the AllReduce as soon as the
                # numerators land; consumers are emitted dead last.
                nc.sync.dma_start(og_in[:], ogsb[:65, :])
                if _os.environ.get("NO_CC") == "1":
                    nc.sync.dma_start(og_out[:], og_in[:])
                else:
                    nc.gpsimd.collective_compute(
                        "AllReduce", ALU.add,
                        replica_groups=[[0, 1, 2, 3], [4, 5, 6, 7]],
                        ins=[og_in.opt()], outs=[og_out.opt()])
                nc.sync.dma_start(ogred[:65, :], og_out[:])

            # ---- band: PV(pr) interleaved with QK(pr+2) ----
            with (
                tc.tile_pool(name="psB", bufs=2, space="PSUM") as psB,
                tc.tile_pool(name="psC", bufs=2, space="PSUM") as psC,
                tc.tile_pool(name="nrm", bufs=2) as nrm,
            ):
                def pv_batch(pr, half, pt, ps, seglist):
                    b = HCOLS * half
                    h = 2 * pr + half
                    vcol = slice((HD + 1) * h, (HD + 1) * h + HD + 1)
                    for kind, qs, w_, ro, t in seglist:
                        if kind == "g":
                            nc.tensor.matmul(
                                ps[:, qs:qs + w_], vsb[0][:, vcol],
                                pt[:, b + PTG0 + ro:b + PTG0 + ro + w_],
                                start=True, stop=False)
                        else:
                            isstop = (t == 7 and qs == 384) or t == 11
                            nc.tensor.matmul(
                                ps[:, qs:qs + w_], vsb[1 + t][:, vcol],
                                pt[:, b + ro:b + ro + w_],
                                start=False, stop=isstop,
                                skip_group_check=not isstop)

                # PV work for one head: global cols first (start), then bands
                PVLIST = ([("g", 0, 512, 0, -1), ("g", 512, 512, 512, -1)]
                          + [("b", qs, w_, ro, t)
                             for (t, qs, w_, ro) in PVSEGS])

                def norm_head(pr, half, ps):
                    # stage numerators+denominator to SBUF (bf16), rank-1
                    # PE-broadcast d over psum rows 0..63, reciprocal from
                    # PSUM (base 0 required), then normalize
                    stg = nrm.tile([65, T], BF16, tag=f"stg{half}",
                                   name=f"stg{half}", bufs=1)
                    nc.vector.tensor_copy(stg[:], ps[:65, :])
                    for c0 in (0, 512):
                        nc.tensor.matmul(
                            ps[0:64, c0:c0 + 512], obc[64:65, :],
                            stg[64:65, c0:c0 + 512],
                            start=True, stop=True, skip_group_check=True)
                    invb = nrm.tile([HD, T], F32, tag=f"invb{half}",
                                    name=f"invb{half}", bufs=1)
                    nc.vector.reciprocal_approx_fast(invb[:], ps[0:64, :])
                    r0 = 64 * half
                    kk = pr % 2
                    nc.vector.tensor_tensor(
                        ctx3[pr // 2][r0:r0 + HD, kk * T:kk * T + T],
                        stg[:HD, :], invb[:], ALU.mult)
                    nc.vector.tensor_tensor(
                        ctxGw[pr][r0:r0 + HD, :],
                        stg[:HD, :G], invb[:, :G], ALU.mult)

                for pr in range(6):
                    # QK of pair pr+2 chunks interleave with PV of pair pr
                    qkc = list(range(6)) if pr + 2 <= 5 else []
                    if qkc:
                        pts[pr + 2] = pt_tile()
                    psh = []
                    for half in (0, 1):
                        ps = psB.tile([65, 1024], F32, tag="pv", name="pv")
                        psh.append(ps)
                    # split PVLIST into 3 batches; alternate h0/h1 per seg
                    nb = 3
                    bsz = (len(PVLIST) + nb - 1) // nb
                    for j in range(nb):
                        if qkc and j < len(qkc):
                            qk_chunk(pr + 2, pts[pr + 2], qkc[j], psC)
                        for seg in PVLIST[j * bsz:(j + 1) * bsz]:
                            for half in (0, 1):
                                pv_batch(pr, half, pts[pr], psh[half], [seg])
                    for j in range(nb, 6):
                        if qkc and j < len(qkc):
                            qk_chunk(pr + 2, pts[pr + 2], qkc[j], psC)
                    for half in (0, 1):
                        norm_head(pr, half, psh[half])

        # ---- output Dense + residual + LayerNorm ----
        with (
            tc.tile_pool(name="wo", bufs=1) as wop,
            tc.tile_pool(name="ln", bufs=2) as lnp,
            tc.tile_pool(name="psD", bufs=2, space="PSUM") as psD,
            tc.tile_pool(name="psE", bufs=1, space="PSUM") as psE,
        ):
            mselr = wop.tile([1, 64], F32, tag="mselr", name="mselr")
            nc.sync.dma_start(mselr[:], dt["mselr"][:])
            wo8t = wop.tile([128, 6 * D], F8, tag="wo8", name="wo8")
            nc.sync.dma_start(wo8t[:], dt["w8"][6])
            wov = wo8t[:].rearrange("p (j i c) -> p j i c", i=2, c=D)
            wob = []
            for k in range(6):
                t_ = wop.tile([128, D], BF16, tag=f"wob{k}", name=f"wob{k}")
                nc.sync.dma_start(t_[:], dt["wob"][128 * k:128 * k + 128, :])
                wob.append(t_)
            epst = wop.tile([128, 1], F32, tag="epst", name="epst")
            nc.gpsimd.memset(epst[:], EPS)
            sumsq = wop.tile([128, 8], F32, tag="sumsq", name="sumsq")
            istd = wop.tile([128, 8], F32, tag="istd", name="istd")
            ycs = [wop.tile([128, D], F32, tag=f"yc{m}", name=f"yc{m}")
                   for m in range(8)]
            c3v = [ctx3[j][:].rearrange("p (k t) -> p k t", t=T)
                   for j in range(3)]

            sstd = wop.tile([128, 8], F32, tag="sstd", name="sstd")
            rs8v = rs8[:].rearrange("p (m c) -> p m c", c=D)

            def ln_stats(m, ys, ps, desc):
                sums = lnp.tile([128, 2], F32, tag="sums", name="sums")
                nc.vector.scalar_tensor_tensor(
                    ys[:], ps[:, :D], desc, rs8v[:, m, :], ALU.mult, ALU.add,
                    accum_out=sums[:, 0:1])
                negmean = lnp.tile([128, 1], F32, tag="negmean",
                                   name="negmean")
                nc.vector.tensor_scalar_mul(negmean[:], sums[:, 0:1],
                                            -1.0 / D)
                nc.vector.tensor_scalar(ycs[m][:], ys[:], negmean[:], None,
                                        ALU.add)
                nc.vector.scalar_tensor_tensor(ys[:], ycs[m][:], 1.0,
                                               ycs[m][:], ALU.mult, ALU.mult,
                                               accum_out=sumsq[:, m:m + 1])

            def ln_finalize(m):
                nc.scalar.activation(sstd[:, m:m + 1], sumsq[:, m:m + 1],
                                     AF.Sqrt, bias=epst[:], scale=1.0 / D)
                nc.vector.reciprocal_approx_fast(istd[:, m:m + 1],
                                                 sstd[:, m:m + 1])
                yo = lnp.tile([128, D], F32, tag="yo", name="yo")
                nc.vector.scalar_tensor_tensor(
                    yo[:], ycs[m][:], istd[:, m:m + 1], vrow["gam"][:],
                    ALU.mult, ALU.mult)
                nc.gpsimd.tensor_tensor(yo[:], yo[:], vrow["bet"][:], ALU.add)
                nc.sync.dma_start(dt["y"][128 * m:128 * m + 128, :], yo[:])

            def ln_pass1(m):
                ys = lnp.tile([128, D], F32, tag="ys", name="ys")
                ps = psD.tile([128, 1024], F32, tag="pd", name="pd")
                for n0, nw in ((0, 512), (512, 256)):
                    for j in range(3):
                        nc.tensor.matmul(
                            ps[:, n0:n0 + nw],
                            c3v[j][:, :, 128 * m:128 * m + 128],
                            wov[:, j, :, n0:n0 + nw],
                            start=(j == 0), stop=(j == 2), perf_mode=DR)
                ln_stats(m, ys, ps, 1.0 / (WSC[6] * 64.0))

            for m in range(1, 8):
                ln_pass1(m)
                ln_finalize(m)

            # ---- fold the AllReduced global-attention output (last) ----
            dstg = wop.tile([1, H * G], F32, tag="dstg", name="dstg")
            nc.vector.tensor_copy(dstg[:], ogred[64:65, :])
            dinvg = wop.tile([1, H * G], F32, tag="dinvg", name="dinvg")
            nc.vector.reciprocal_approx_fast(dinvg[:], dstg[:])
            # broadcast msel0/d across 64 partitions on the PE (fp32 rank-1)
            fps = psE.tile([64, H * G], F32, tag="fps", name="fps")
            for f0 in range(0, H * G, 512):
                nc.tensor.matmul(fps[:, f0:f0 + 512], mselr[0:1, :],
                                 dinvg[:, f0:f0 + 512], start=True, stop=True,
                                 skip_group_check=True)
            for hh in range(12):
                pr, half = hh // 2, hh % 2
                r0 = 64 * half
                t1 = lnp.tile([128, G], F32, tag="ft1", name="ft1")
                nc.vector.tensor_tensor(
                    t1[r0:r0 + 64, :], ogred[:64, G * hh:G * hh + G],
                    fps[:, G * hh:G * hh + G], ALU.mult)
                nc.vector.tensor_scalar_mul(
                    ctxG[pr][r0:r0 + 64, :], ctxGw[pr][r0:r0 + 64, :],
                    msel[r0:r0 + 64, 1:2])
                nc.gpsimd.tensor_tensor(
                    ctxG[pr][r0:r0 + 64, :], ctxG[pr][r0:r0 + 64, :],
                    t1[r0:r0 + 64, :], ALU.add)

            # ---- m=0 tile: bf16 Dense off ctxG, then its LayerNorm ----
            ys0 = lnp.tile([128, D], F32, tag="ys", name="ys")
            ps0 = psD.tile([128, 1024], F32, tag="pd", name="pd")
            for n0, nw in ((0, 512), (512, 256)):
                for k in range(6):
                    nc.tensor.matmul(
                        ps0[:, n0:n0 + nw], ctxG[k][:, :],
                        wob[k][:, n0:n0 + nw],
                        start=(k == 0), stop=(k == 5))
            ln_stats(0, ys0, ps0, 1.0 / 64.0)
            ln_finalize(0)
            if "dbg_ctx" in dt:
                for j in range(3):
                    nc.sync.dma_start(dt["dbg_ctx"][j], ctx3[j][:])
                nc.sync.dma_start(dt["dbg_og"][:], ogred[:65, :])
        ogd_cm.__exit__(None, None, None)


def build_nc():
    nc = bacc.Bacc(trn_type="TRN2", num_devices=8)
    dt = {
        "xa": nc.dram_tensor("xa", [128, 6 * KT], F8, kind="ExternalInput"),
        "xb": nc.dram_tensor("xb", [128, 6 * KT], F8, kind="ExternalInput"),
        "w8": nc.dram_tensor("w8", [7, 128, 6 * D], F8, kind="ExternalInput"),
        "wob": nc.dram_tensor("wob", [D, D], BF16, kind="ExternalInput"),
        "res": nc.dram_tensor("res", [T, D], BF16, kind="ExternalInput"),
        "maskm": nc.dram_tensor("maskm", [128, BAND_COLS], BF16,
                                kind="ExternalInput"),
        "msel": nc.dram_tensor("msel", [128, 2], F32, kind="ExternalInput"),
        "mselr": nc.dram_tensor("mselr", [1, 64], F32, kind="ExternalInput"),
        "vrep": nc.dram_tensor("vrep", [4, 128, D], BF16, kind="ExternalInput"),
        "biasT": nc.dram_tensor("biasT", [128, 24], F32, kind="ExternalInput"),
        "y": nc.dram_tensor("y", [T, D], F32, kind="ExternalOutput"),
    }
    if _os.environ.get("DBG") == "1":
        dt["dbg_ctx"] = nc.dram_tensor("dbg_ctx", [3, 128, 2 * T], F8,
                                       kind="ExternalOutput")
        dt["dbg_og"] = nc.dram_tensor("dbg_og", [65, H * G], BF16,
                                      kind="ExternalOutput")
    with tile.TileContext(nc) as tc:
        _emit(tc, dt)
    nc.compile()
    return nc


def host_inputs(inputs):
    """Build the 8 per-core input maps from the full problem inputs."""
    hs = np.asarray(inputs["hidden_states"], np.float32)
    assert hs.shape == (B, S, D)
    E4 = ml_dtypes.float8_e4m3
    bf = lambda a: np.ascontiguousarray(np.asarray(a, np.float32)).astype(
        ml_dtypes.bfloat16)
    f32 = lambda a: np.ascontiguousarray(np.asarray(a, np.float32))

    def pair8(a2d, ncols):
        # [768, ncols] fp8 -> [128, 3, 2, ncols] -> [128, 6*ncols]
        return np.ascontiguousarray(
            a2d.reshape(3, 2, 128, ncols).transpose(2, 0, 1, 3)
            .reshape(128, 6 * ncols))

    wstack = np.stack([
        np.asarray(inputs["Wq"], np.float32) * SCALE,
        np.asarray(inputs["Wk"], np.float32),
        np.asarray(inputs["Wv"], np.float32),
        np.asarray(inputs["Wkg"], np.float32),
        np.asarray(inputs["Wvg"], np.float32),
        np.asarray(inputs["Wqg"], np.float32) * SCALE,
        np.asarray(inputs["Wo"], np.float32),
    ])
    w8 = np.stack([
        pair8((wstack[i] * WSC[i]).astype(E4), D) for i in range(7)])
    wob = bf(wstack[6])

    vecs = np.stack([
        np.asarray(inputs["bq"], np.float32) * SCALE,
        np.asarray(inputs["bk"], np.float32),
        np.asarray(inputs["bkg"], np.float32),
        np.asarray(inputs["bqg"], np.float32) * SCALE,
    ])
    bo = np.asarray(inputs["bo"], np.float32)
    biasT = np.zeros((128, 24), np.float32)
    for row in range(4):
        for p in range(6):
            biasT[:, row * 6 + p] = vecs[row, 128 * p:128 * p + 128]
    vrep = bf(np.broadcast_to(
        np.stack([
            np.asarray(inputs["bv"], np.float32),
            np.asarray(inputs["bvg"], np.float32),
            np.asarray(inputs["ln_gamma"], np.float32),
            np.asarray(inputs["ln_beta"], np.float32),
        ])[:, None, :], (4, 128, D)))

    in_maps = []
    for c in range(8):
        b, j = c // 4, c % 4
        r0 = j * T
        x = hs[b]
        xp = np.zeros((S + 2 * W, D), np.float32)
        xp[W:W + S] = x
        x_kv = np.concatenate([x[:G], xp[r0:r0 + HALO]], axis=0)  # [1664, D]
        xT = np.ascontiguousarray(x_kv.T)                          # [768, 1664]
        xa8 = xT.astype(E4)
        xb8 = (xT - xa8.astype(np.float32)).astype(E4)
        res = bf(x[r0:r0 + T] + bo)

        # multiplicative band mask, {0,1} bf16, MOFF layout
        mask = np.zeros((128, BAND_COLS), np.float32)
        for t in range(NBT):
            lo = _lo(t)
            nq = NQ[t]
            jj = np.arange(128 * t, 128 * t + 128)[:, None]
            ii = np.arange(lo * 128, lo * 128 + nq)[None, :]
            kpos = r0 - W + jj
            valid = ((jj - ii >= 0) & (jj - ii <= 2 * W)
                     & (kpos >= G) & (kpos < S))
            mask[:, MOFF[t]:MOFF[t] + nq] = valid.astype(np.float32)

        m = 1.0 if j == 0 else 0.0
        msel = np.zeros((128, 2), np.float32)
        msel[:, 0] = m
        msel[:, 1] = 1.0 - m

        in_maps.append({
            "xa": pair8(xa8, KT), "xb": pair8(xb8, KT),
            "w8": w8, "wob": wob, "res": res,
            "maskm": bf(mask), "msel": f32(msel),
            "mselr": np.full((1, 64), m, np.float32),
            "vrep": vrep, "biasT": biasT,
        })
    return in_maps


_NC_CACHE = {}


def _get_nc():
    if "nc" not in _NC_CACHE:
        _NC_CACHE["nc"] = build_nc()
    return _NC_CACHE["nc"]


def kernel(**inputs) -> np.ndarray:
    # sanity-check the fixed global-attention pattern this kernel hardcodes
    iga = np.asarray(inputs["is_index_global_attn"])
    assert iga.shape == (B, S)
    expect = np.broadcast_to(np.arange(S) < G, (B, S))
    assert np.array_equal(iga, expect), "kernel hardcodes a G=128 prefix"
    am = np.asarray(inputs["attention_mask"], np.float32)
    assert np.all(am == 0.0), "kernel assumes no key-padding mask"

    nc = _get_nc()
    in_maps = host_inputs(inputs)
    res = bass_utils.run_bass_kernel_spmd(nc, in_maps, core_ids=list(range(8)))
    outs = res.results if hasattr(res, "results") else res
    y = np.zeros((B, S, D), np.float32)
    for c in range(8):
        b, j = c // 4, c % 4
        y[b, j * T:(j + 1) * T] = outs[c]["y"]
    return y


if __name__ == "__main__":
    nc = build_nc()
    print("build ok; instructions:",
          sum(len(bb.instructions) for bb in nc.main_func.blocks))
